# revision 114
# baseline (speedup 1.0000x reference)
"""Causal self-attention (value-residual + QK RMSNorm + RoPE + sigmoid gate)
Trainium2 Bass kernel, sharded over 8 NeuronCores.

Sharding: core c handles batch b = c // 4 and the 4 heads [4*(c%4), 4*(c%4)+4).
Each core computes its heads' QKV, attention and gating, then a partial
c_proj contribution out_partial^T = Wproj[:, islice] @ y_gated^T  [1024, 2048]
in f16; host sums the 4 partials per batch in f32 and transposes back.

Key optimizations over the v1 kernel:
 - QKV projections in fp8 (e4m3) DoubleRow matmuls with 3-term residual
   correction: q = x8@W8 + x8@Wr8 + xr8@W8 (likewise k, v).  Host prepares
   x8/xr8 and W8/Wr8; PE cost drops to 0.75x of the f16 path at full
   precision recovery (error ~ quantization^2).
 - Phase interleaving: attention for q-chunk qc only needs the first
   4*(qc+1) token tiles of phase 1, so phase-1 m-groups are emitted
   between attention chunks and all five engines stay busy.
 - Engine rebalance: PSUM->SBUF copies and v-mix on Pool (gpsimd), RMS
   stats/RoPE/masks/y-scale on DVE, exp-only on ACT, output DMAs issued
   from the ACT queue, input DMAs from SP.
 - y path in bf16 (halves y-transpose cost), c_proj keeps bf16 weights.
 - One fused input DMA per token tile (x8|xr8) plus one for (v1|cos|sin).
 - 4-head y accumulator in one PSUM bank with batched normalization
   (one reciprocal + one gate*rec + one apply per j instead of four).
 - Paired PSUM drains: both transpose halves share one psum tile and are
   drained by a single 256-wide copy; cproj's two half-rounds accumulate
   into one full-bank psum tile with a single 512-wide drain; the four
   gate matmuls share one psum tile (pending-zero writes) with one drain.
 - All cproj/ytp drains on DVE (ACT runs exps only in the steady state);
   early qk-transpose drains (m<12) ride the then-idle ACT.
 - f16 gate path and f16 output staging/DMA (host sums partials in f32).
"""

import sys

sys.path.insert(0, "/opt/trn_rl_repo")

import math

import numpy as np
import ml_dtypes

import concourse.bass as bass
import concourse.mybir as mybir
import concourse.tile as tile
from concourse.bass_utils import run_bass_kernel_spmd
from concourse import bacc


# Force Exp and Ln to resolve to the combined natural_log_exp_and_others set:
# the greedy table-load pass otherwise alternates exp_and_others/natural_log,
# inserting a table load per switch. Entry positions are preserved so
# act_func_set_id indices stay valid.
_orig_gat = bacc.get_activation_tables


def _gat_combined(arch):
    out = {}
    for name, fns in _orig_gat(arch).items():
        if name != "natural_log_exp_and_others":
            fns = {f for f in fns
                   if str(f).split(".")[-1] not in ("Exp", "Ln")}
        out[name] = fns
    return out


bacc.get_activation_tables = _gat_combined

F32 = mybir.dt.float32
F32R = mybir.dt.float32r
F16 = mybir.dt.float16
BF16 = mybir.dt.bfloat16
F8 = mybir.dt.float8e4
AF = mybir.ActivationFunctionType
OP = mybir.AluOpType
DR = mybir.MatmulPerfMode.DoubleRow

B, T, D, H, HD = 2, 2048, 1024, 16, 64
HL = 4            # heads per core
OL = HL * HD      # 256 local qkv width
NT = T // 128     # 16 t-tiles
KD = D // 128     # 8 contraction tiles
NQC = T // 512    # 4 q chunks
EPS = 1.1920929e-7
ATTN_SCALE = 0.1
ROPE_BASE = 10000.0
N_CORES = 8
NP8 = ml_dtypes.float8_e4m3
SX = 32.0         # fp8 scale for x8/xr8 (e4m3 max is 240; |x| < 6)
SW = 64.0         # fp8 scale for W8/Wr8 (|W| <= 1/32)
DEQ = 1.0 / (SX * SW)

_CACHE = {}
PHASE_MARKS = []   # (label, first_instruction_number) in emission order


def _mark(nc, label):
    n = nc._state.get_next_instruction_name()  # consumes one name
    PHASE_MARKS.append((label, int(n.split("-")[1])))


def r32(ap):
    return ap.bitcast(F32R)


def build_program():
    nc = bacc.Bacc("TRN2", target_bir_lowering=False, debug=False, num_devices=1)

    # host-tiled inputs (layouts chosen for single fat DMAs, no elem penalty)
    xx8 = nc.dram_tensor("xx8", [128, NT, 2 * KD * 128], F8,
                         kind="ExternalInput").ap()          # x8 | xr8
    vcs = nc.dram_tensor("vcs", [128, NT, 384], F16,
                         kind="ExternalInput").ap()          # lam*v1 | cos | sin
    WSZ = (KD // 2) * 2 * OL
    wq8 = nc.dram_tensor("wq8", [128, WSZ], F8, kind="ExternalInput").ap()
    wk8 = nc.dram_tensor("wk8", [128, WSZ], F8, kind="ExternalInput").ap()
    wv8 = nc.dram_tensor("wv8", [128, WSZ], F8, kind="ExternalInput").ap()
    wqr8 = nc.dram_tensor("wqr8", [128, WSZ], F8, kind="ExternalInput").ap()
    wkr8 = nc.dram_tensor("wkr8", [128, WSZ], F8, kind="ExternalInput").ap()
    wvr8 = nc.dram_tensor("wvr8", [128, WSZ], F8, kind="ExternalInput").ap()
    wp = nc.dram_tensor("wp", [128, 2, D], BF16, kind="ExternalInput").ap()
    wg = nc.dram_tensor("wg", [12, HL], F16, kind="ExternalInput").ap()
    xg = nc.dram_tensor("xg", [12, T], F16, kind="ExternalInput").ap()
    id16 = nc.dram_tensor("id16", [128, 128], F16, kind="ExternalInput").ap()
    idbf = nc.dram_tensor("idbf", [128, 128], BF16, kind="ExternalInput").ap()
    msk16 = nc.dram_tensor("msk16", [128, 128], F16, kind="ExternalInput").ap()
    outT = nc.dram_tensor("outT", [D, T], F16, kind="ExternalOutput").ap()

    with tile.TileContext(nc) as tc:
        import contextlib
        stack = contextlib.ExitStack()
        pers = stack.enter_context(tc.tile_pool(name="pers", bufs=1))
        xp = stack.enter_context(tc.tile_pool(name="xp", bufs=5))
        sp = stack.enter_context(tc.tile_pool(name="sp", bufs=4))
        pp = stack.enter_context(tc.tile_pool(name="pp", bufs=50))
        op_ = stack.enter_context(tc.tile_pool(name="op", bufs=10))
        yp = stack.enter_context(tc.tile_pool(name="yp", bufs=2))
        # PSUM budget is 8 banks of 2KB; every ring slot is bank-rounded.
        # pss: 2 slots x 2 banks.  psa: tag "ph1" (qkv/transpose/gate psums)
        # and tag "attn" (y/ytranspose/cproj psums), 2 bank-slots each.
        pss = stack.enter_context(tc.tile_pool(name="pss", bufs=2, space="PSUM"))
        psa = stack.enter_context(tc.tile_pool(name="psa", bufs=2, space="PSUM"))

        # ---- persistent tiles / weight DMAs (issue order = need order) ----
        def ptile(shape, dtype, name):
            return pers.tile(shape, dtype, name=name, tag=name, bufs=1)

        def wview(t):
            return t.rearrange("p (c s o) -> p c s o", s=2, o=OL)

        wq8_f = ptile([128, WSZ], F8, "wq8_sb")
        wqr8_f = ptile([128, WSZ], F8, "wqr8_sb")
        wk8_f = ptile([128, WSZ], F8, "wk8_sb")
        wkr8_f = ptile([128, WSZ], F8, "wkr8_sb")
        wv8_f = ptile([128, WSZ], F8, "wv8_sb")
        wvr8_f = ptile([128, WSZ], F8, "wvr8_sb")
        nc.sync.dma_start(out=wq8_f, in_=wq8)
        nc.sync.dma_start(out=wqr8_f, in_=wqr8)

        def kv_dma_hook(nm):
            # called during phase1(0) emission, before the k / v matmuls
            if nm == "k":
                nc.sync.dma_start(out=wk8_f, in_=wk8)
                nc.sync.dma_start(out=wkr8_f, in_=wkr8)
            elif nm == "v":
                nc.sync.dma_start(out=wv8_f, in_=wv8)
                nc.sync.dma_start(out=wvr8_f, in_=wvr8)
        wq8_sb, wqr8_sb = wview(wq8_f), wview(wqr8_f)
        wk8_sb, wkr8_sb = wview(wk8_f), wview(wkr8_f)
        wv8_sb, wvr8_sb = wview(wv8_f), wview(wvr8_f)
        id16_sb = ptile([128, 128], F16, "id16_sb")
        idbf_sb = ptile([128, 128], BF16, "idbf_sb")
        msk_sb = ptile([128, 128], F16, "msk_sb")
        xg_sb = ptile([12, T], F16, "xg_sb")
        wg_sb = ptile([12, HL], F16, "wg_sb")
        nc.sync.dma_start(out=id16_sb, in_=id16)
        wp_sb = ptile([128, 2, D], BF16, "wp_sb")

        def late_dmas():
            nc.sync.dma_start(out=msk_sb, in_=msk16)
            nc.sync.dma_start(out=xg_sb, in_=xg)
            nc.sync.dma_start(out=wg_sb, in_=wg)
            nc.sync.dma_start(out=idbf_sb, in_=idbf)
            nc.sync.dma_start(out=wp_sb, in_=wp)

        eps_sb = ptile([128, 1], F32, "eps_sb")
        nc.vector.memset(eps_sb, EPS)

        v_m = ptile([128, NT, HL, 66], F16, "v_m")
        nc.gpsimd.memset(v_m[:, :, :, 64:66], 1.0)
        gate_all = ptile([128, NT, HL], F32, "gate_all")
        kTt = ptile([128, NT, 2, 128], F16, "kTt")
        qTc = ptile([128, NQC, 2, 512], F16, "qTc")
        ysb = ptile([128, NT, 2, 128], BF16, "ysb")

        rots = [None] * NT

        pend1 = {}

        # ---- phase 1a for one token tile m -------------------------------
        # part="h": DMAs/QKV/drains/RMS-stat only; part="t": normalize+RoPE.
        # Splitting the first tiles hides the ACT ln/exp round-trip in the
        # otherwise-empty startup DVE queue.
        def phase1(m, part="all"):
            if part == "t":
                qks, ctv, stv = pend1.pop(m)
                phase1_tail(m, qks, ctv, stv)
                return
            xt = xp.tile([128, 2 * KD * 128], F8, name=f"xx{m}", tag="xx")
            nc.sync.dma_start(out=xt, in_=xx8[:, m])
            vt = xp.tile([128, 384], F16, name=f"vcs{m}", tag="vcs")
            nc.sync.dma_start(out=vt, in_=vcs[:, m])
            x8v = xt[:, 0:1024].rearrange("p (c s t) -> p c s t", s=2, t=128)
            xr8v = xt[:, 1024:2048].rearrange("p (c s t) -> p c s t", s=2, t=128)
            v1t = vt[:, 0:256].rearrange("p (h d) -> p h d", h=HL)
            ctv = (vt[:, 256:320].rearrange("p (a f) -> p a f", a=2)
                   .unsqueeze(2).broadcast_to([128, 2, HL, 32]))
            stv = (vt[:, 320:384].rearrange("p (a f) -> p a f", a=2)
                   .unsqueeze(2).broadcast_to([128, 2, HL, 32]))

            qk_ps = psa.tile([128, 2, OL], F32, name=f"qk_ps{m}", tag="ph1")
            pses = []
            for w8, wr8, nm in ((wq8_sb, wqr8_sb, "q"), (wk8_sb, wkr8_sb, "k"),
                                (wv8_sb, wvr8_sb, "v")):
                if m == 0 and nm != "q":
                    kv_dma_hook(nm)
                if nm == "q":
                    ps = qk_ps[:, 0]
                elif nm == "k":
                    ps = qk_ps[:, 1]
                else:
                    ps = psa.tile([128, OL], F32, name=f"v_ps{m}", tag="ph1")
                nc.tensor.matmul(ps, x8v[:, 0], w8[:, 0], start=True,
                                 stop=False, perf_mode=DR)
                for c in range(1, 4):
                    nc.tensor.matmul(ps, x8v[:, c], w8[:, c], start=False,
                                     stop=False, perf_mode=DR)
                for c in range(4):
                    nc.tensor.matmul(ps, x8v[:, c], wr8[:, c], start=False,
                                     stop=False, perf_mode=DR)
                for c in range(4):
                    nc.tensor.matmul(ps, xr8v[:, c], w8[:, c], start=False,
                                     stop=(c == 3), perf_mode=DR)
                pses.append(ps)
            q_ps, k_ps, v_ps = pses

            # PSUM carries x8@W8-style products scaled by SX*SW = 2048.
            # DVE dequant-copies drain PSUM (Pool cannot touch PSUM, and a
            # DVE op may read at most one PSUM operand).
            qks = sp.tile([128, 2, HL, HD], F16, name=f"qks{m}", tag="qks")
            nc.vector.tensor_scalar_mul(
                qks, qk_ps.rearrange("t a (h d) -> t a h d", h=HL), DEQ)
            nc.vector.scalar_tensor_tensor(
                out=v_m[:, m, :, 0:64],
                in0=v_ps.rearrange("t (h d) -> t h d", h=HL),
                scalar=DEQ, in1=v1t, op0=OP.mult, op1=OP.add)

            qkf = qks.rearrange("t a h d -> t (a h d)")
            scr = sp.tile([128, 2 * OL], F16, name=f"scr{m}", tag="scr")
            nc.vector.tensor_mul(scr, qkf, qkf)
            ss = sp.tile([128, 8], F16, name=f"ss{m}", tag="ss")
            with nc.allow_low_precision(reason="64-term f16 sum feeds rsqrt"):
                nc.vector.tensor_reduce(
                    out=ss, in_=scr.rearrange("t (g d) -> t g d", d=HD),
                    axis=mybir.AxisListType.X, op=OP.add)
            lnv = sp.tile([128, 8], F32, name=f"lnv{m}", tag="lnv")
            nc.scalar.activation(lnv, ss, AF.Ln, scale=1.0 / HD, bias=eps_sb)
            rs = sp.tile([128, 2, HL], F16, name=f"rs{m}", tag="rs")
            # rs = rsqrt(mean q^2); 0.1 for q is folded into host cos/sin.
            nc.scalar.activation(rs.rearrange("t a h -> t (a h)"), lnv,
                                 AF.Exp, scale=-0.5)
            rs2 = sp.tile([128, 2, HL, 2], F16, name=f"rs2{m}", tag="rs2")
            nc.gpsimd.tensor_copy(
                rs2, rs.unsqueeze(3).broadcast_to([128, 2, HL, 2]))
            if part == "h":
                pend1[m] = (qks, ctv, stv)
                pend1[(m, "rs2")] = rs2
                return
            phase1_tail(m, qks, ctv, stv, rs2)

        def phase1_tail(m, qks, ctv, stv, rs2=None):
            if rs2 is None:
                rs2 = pend1.pop((m, "rs2"))
            qk = sp.tile([128, 2, HL, HD], F16, name=f"qk{m}", tag="qk")
            nc.vector.tensor_mul(
                qk.rearrange("t a h (f e) -> t a h f e", e=2), qks.rearrange(
                    "t a h (f e) -> t a h f e", e=2),
                rs2.unsqueeze(3).broadcast_to([128, 2, HL, 32, 2]))

            a_, b_ = qk[:, :, :, 0:32], qk[:, :, :, 32:64]
            rot = sp.tile([128, 2, HL, HD], F16, name=f"rot{m}", tag="rot",
                          bufs=3)
            r1, r2 = rot[:, :, :, 0:32], rot[:, :, :, 32:64]
            tmp = sp.tile([128, 2, HL, 32], F16, name=f"tmp{m}", tag="tmp")
            tmp2 = sp.tile([128, 2, HL, 32], F16, name=f"tmp2{m}", tag="tmp2")
            nc.vector.tensor_mul(tmp, b_, stv)
            nc.vector.tensor_mul(r1, a_, ctv)
            nc.vector.tensor_add(r1, r1, tmp)
            nc.gpsimd.tensor_mul(tmp2, a_, stv)
            nc.vector.tensor_mul(r2, b_, ctv)
            nc.vector.tensor_sub(r2, r2, tmp2)

            rots[m] = rot

        # ---- phase 1b: transposes for tile m (emitted later so they do
        # not block the next tile's QKV matmuls in the in-order PE queue) --
        def phase1b(m):
            # early tiles drain their transposes on the then-idle ACT;
            # both halves share one psum tile -> ONE 256-wide drain copy
            cp = nc.scalar.copy if m < 12 else nc.vector.tensor_copy
            rotf = rots[m].rearrange("t a h d -> t (a h d)")  # q 0-255 k 256-511
            tps = psa.tile([128, 2, 128], F16, name=f"tpq{m}", tag="attn")
            for half in range(2):
                nc.tensor.transpose(tps[:, half],
                                    rotf[:, 128 * half:128 * half + 128],
                                    id16_sb)
            cp(qTc[:, m // 4, :, 128 * (m % 4):128 * (m % 4) + 128], tps)
            tpk = psa.tile([128, 2, 128], F16, name=f"tpk{m}", tag="attn")
            for half in range(2):
                nc.tensor.transpose(tpk[:, half],
                                    rotf[:, 256 + 128 * half:384 + 128 * half],
                                    id16_sb)
            cp(kTt[:, m], tpk)

        # ---- gate for a group of 4 token tiles ---------------------------
        def gate_group(g):
            gg = sp.tile([128, 4, HL], F32, name=f"gg{g}", tag="gg")
            g_ps = psa.tile([128, 4, HL], F32, name=f"g_ps{g}", tag="attn")
            # one zero-region group; starts mark the region pending-zero so
            # each disjoint [*, mi] write replaces rather than accumulates
            for mi in range(4):
                m = 4 * g + mi
                nc.tensor.matmul(g_ps[:, mi], xg_sb[:, 128 * m:128 * m + 128],
                                 wg_sb, start=(mi == 0), stop=(mi == 3))
            nc.vector.tensor_copy(gg, g_ps)
            gge = sp.tile([128, 4, HL], F32, name=f"gge{g}", tag="gge")
            nc.scalar.activation(gge.rearrange("t a h -> t (a h)"),
                                 gg.rearrange("t a h -> t (a h)"),
                                 AF.Exp, scale=-1.0)
            nc.gpsimd.tensor_scalar_add(gge, gge, 1.0)
            nc.vector.reciprocal(
                gate_all[:, 4 * g:4 * g + 4].rearrange("t a h -> t (a h)"),
                gge.rearrange("t a h -> t (a h)"))

        # ---- attention for one q chunk, split into schedulable blocks ----
        pts_all = [[[None] * NT for _ in range(2)] for _ in range(NQC)]

        def att_scores(qc, i0=0, i1=None):
            pts = pts_all[qc]
            for i in range(i0, 4 * qc + 4 if i1 is None else i1):
                ql0 = max(0, 128 * (i - 4 * qc))
                ncols = 512 - ql0
                for p in range(2):
                    s_ps = pss.tile([128, 2, 512], F32, name=f"s{p}_{qc}_{i}",
                                    tag="pss")
                    for r in range(2):
                        prt = slice(64 * r, 64 * r + 64)
                        nc.tensor.matmul(
                            s_ps[:, r, 0:ncols],
                            kTt[prt, i, p],
                            qTc[prt, qc, p, ql0:512],
                            start=True, stop=True,
                            tile_position=(64 * r, 0))
                    pt = pp.tile([128, 2, 512], F16, name=f"pt{p}_{qc}_{i}",
                                 tag="pt")
                    nc.scalar.activation(pt[:, :, ql0:512], s_ps[:, :, 0:ncols],
                                         AF.Exp)
                    if i >= 4 * qc:
                        mb = msk_sb.unsqueeze(1).broadcast_to([128, 2, 128])
                        mul = nc.vector.tensor_mul if qc >= 2 \
                            else nc.gpsimd.tensor_mul
                        mul(pt[:, :, ql0:ql0 + 128],
                            pt[:, :, ql0:ql0 + 128], mb)
                    pts[p][i] = pt

        def att_pv(qc, js0=0, js1=4):
            # 4-head y accumulator in ONE psum bank; batched normalize:
            # rec4 = 1/rowsum (DVE), rg = rec*gate (Pool), one apply (DVE).
            pts = pts_all[qc]
            for js in range(js0, js1):
                j = 4 * qc + js
                y_ps = psa.tile([128, HL, 66], F32, name=f"y_{j}", tag="attn")
                for p in range(2):
                    for r in range(2):
                        h = 2 * p + r
                        for i in range(j + 1):
                            nc.tensor.matmul(
                                y_ps[:, h],
                                pts[p][i][:, r, 128 * js:128 * js + 128],
                                v_m[:, i, h, 0:66],
                                start=(i == 0), stop=(i == j))
                rec4 = sp.tile([128, HL], F32, name=f"rec{j}", tag="rec")
                nc.vector.reciprocal(
                    rec4, y_ps[:, :, 64:65].rearrange("t h o -> t (h o)"))
                rg = sp.tile([128, HL], F32, name=f"rg{j}", tag="rg")
                rgmul = nc.vector.tensor_mul if qc == 3 \
                    else nc.gpsimd.tensor_mul
                rgmul(rg, rec4, gate_all[:, j])
                ysb_j = (ysb[:, j].rearrange("t p d -> t (p d)")
                         .rearrange("t (h e) -> t h e", h=HL))
                nc.vector.tensor_mul(
                    ysb_j, y_ps[:, :, 0:64],
                    rg.unsqueeze(2).broadcast_to([128, HL, 64]))

        yTq_c = [None] * NQC
        ots = [None] * 8

        def ensure_yT(qc):
            if yTq_c[qc] is None:
                yTq_c[qc] = yp.tile([128, 2, 512], BF16, name=f"yT{qc}",
                                    tag="yT")
            return yTq_c[qc]

        def att_ytp(qc, js):
            # PE-transpose ysb[j] into yTq columns (+DVE drains)
            yTq = ensure_yT(qc)
            j = 4 * qc + js
            for half in range(2):
                ytp = psa.tile([128, 128], BF16, name=f"ty{j}{half}",
                               tag="attn")
                nc.tensor.transpose(ytp, ysb[:, j, half], idbf_sb)
                nc.vector.tensor_copy(yTq[:, half, 128 * js:128 * js + 128],
                                      ytp)

        def att_pv_j(qc, js, i0, i1, stop):
            # partial PV chain for tier js: i in [i0, i1); psum group left
            # open unless stop. Caller finishes with att_pv_fin.
            pts = pts_all[qc]
            j = 4 * qc + js
            if i0 == 0:
                pv_ps[(qc, js)] = psa.tile([128, HL, 66], F32,
                                           name=f"y_{j}", tag="ph1")
            y_ps = pv_ps[(qc, js)]
            for p in range(2):
                for r in range(2):
                    h = 2 * p + r
                    for i in range(i0, i1):
                        nc.tensor.matmul(
                            y_ps[:, h],
                            pts[p][i][:, r, 128 * js:128 * js + 128],
                            v_m[:, i, h, 0:66],
                            start=(i == 0), stop=(stop and i == i1 - 1))

        def att_pv_fin(qc, js):
            j = 4 * qc + js
            y_ps = pv_ps[(qc, js)]
            rec4 = sp.tile([128, HL], F32, name=f"rec{j}", tag="rec")
            nc.vector.reciprocal(
                rec4, y_ps[:, :, 64:65].rearrange("t h o -> t (h o)"))
            rg = sp.tile([128, HL], F32, name=f"rg{j}", tag="rg")
            nc.gpsimd.tensor_mul(rg, rec4, gate_all[:, j])
            ysb_j = (ysb[:, j].rearrange("t p d -> t (p d)")
                     .rearrange("t (h e) -> t h e", h=HL))
            nc.vector.tensor_mul(
                ysb_j, y_ps[:, :, 0:64],
                rg.unsqueeze(2).broadcast_to([128, HL, 64]))

        pv_ps = {}

        def att_cproj(qc, tc2, dma):
            yTq = ensure_yT(qc)
            lsl = slice(256 * tc2, 256 * tc2 + 256)
            tsl = slice(512 * qc, 512 * qc + 512)
            for oc in range(8):
                osl = slice(128 * oc, 128 * oc + 128)
                if tc2 == 0:
                    ots[oc] = op_.tile([128, 512], F16,
                                       name=f"ot{oc}_{qc}", tag="ot")
                pr = psa.tile([128, 256], F32, name=f"pr{oc}_{qc}_{tc2}",
                              tag="attn")
                for a in range(2):
                    nc.tensor.matmul(pr, wp_sb[:, a, osl],
                                     yTq[:, a, lsl],
                                     start=(a == 0), stop=(a == 1))
                nc.vector.tensor_copy(ots[oc][:, lsl], pr)
                if dma:
                    nc.sync.dma_start(out=outT[osl, tsl], in_=ots[oc])

        def att_tail(qc, part=None):
            # tail drains ride ACT in round 0/1 (ACT slack); DVE otherwise.
            # qc3 (post-last-exp, ACT idle): alternate copies ACT/DVE so the
            # final drain runs on both engines in parallel.
            on_act = False
            ot_copy = nc.scalar.copy if on_act else nc.vector.tensor_copy
            ytp_copy = nc.vector.tensor_copy
            if yTq_c[qc] is None:
                yTq_c[qc] = yp.tile([128, 2, 512], BF16, name=f"yT{qc}",
                                    tag="yT")
            yTq = yTq_c[qc]
            parts = (0, 1) if part is None else (part,)
            for tc2 in parts:
                for js in (2 * tc2, 2 * tc2 + 1):
                    j = 4 * qc + js
                    ytp = psa.tile([128, 2, 128], BF16, name=f"ty{j}",
                                   tag="attn")
                    for half in range(2):
                        nc.tensor.transpose(ytp[:, half], ysb[:, j, half],
                                            idbf_sb)
                    ytp_copy(yTq[:, :, 128 * js:128 * js + 128], ytp)
            if part is None:
                tsl = slice(512 * qc, 512 * qc + 512)
                for oc in range(8):
                    osl = slice(128 * oc, 128 * oc + 128)
                    ot = op_.tile([128, 512], F16, name=f"ot{oc}_{qc}",
                                  tag="ot")
                    # qc3: odd oc borrow the idle ph1 psum ring -> 4
                    # cproj rounds in flight instead of 2 at the very end
                    ptag = "ph1" if (qc == 3 and oc % 2 == 1) else "attn"
                    pr = psa.tile([128, 512], F32, name=f"pr{oc}_{qc}",
                                  tag=ptag)
                    for tc2 in range(2):
                        lsl = slice(256 * tc2, 256 * tc2 + 256)
                        for a in range(2):
                            nc.tensor.matmul(pr[:, lsl], wp_sb[:, a, osl],
                                             yTq[:, a, lsl],
                                             start=(a == 0), stop=(a == 1))
                    cp = (nc.scalar.copy if (qc == 3 and oc % 2 == 1)
                          else ot_copy)
                    cp(ot, pr)
                    nc.sync.dma_start(out=outT[osl, tsl], in_=ot)
            else:
                # half-tail: fill ot[:, half]; the tc2=1 half emits the DMA
                tc2 = part
                lsl = slice(256 * tc2, 256 * tc2 + 256)
                tsl = slice(512 * qc, 512 * qc + 512)
                for oc in range(8):
                    osl = slice(128 * oc, 128 * oc + 128)
                    if tc2 == 0:
                        ots[oc] = op_.tile([128, 512], F16,
                                           name=f"ot{oc}_{qc}", tag="ot")
                    pr = psa.tile([128, 256], F32, name=f"pr{oc}_{qc}_{tc2}",
                                  tag="attn")
                    for a in range(2):
                        nc.tensor.matmul(pr, wp_sb[:, a, osl],
                                         yTq[:, a, lsl],
                                         start=(a == 0), stop=(a == 1))
                    ot_copy(ots[oc][:, lsl], pr)
                    if tc2 == 1:
                        nc.sync.dma_start(out=outT[osl, tsl], in_=ots[oc])

        # ---- schedule (software-pipelined; per-engine queues are in-order
        # so later-dependent PE work is always emitted after independent
        # PE work that can run now) ----------------------------------------
        # ---- schedule: phase-1 tiles staggered so two QKV tile-loads run
        # ahead of the chain, and the next group's first tiles are issued
        # before each score round to keep PE fed during chain tails. -------
        sched = """
        phase1(0, "h")
        phase1(1, "h")
        late_dmas()
        phase1(0, "t"); phase1(2, "h")
        phase1b(0); phase1(1, "t"); phase1(3, "h")
        phase1b(1); phase1(2, "t")
        phase1b(2); phase1(3, "t"); gate_group(0)
        phase1(4); phase1b(3)
        att_scores(0, 0, 1); phase1(5)
        att_scores(0, 1, 2); phase1b(4)
        att_scores(0, 2, 3); phase1(6)
        att_scores(0, 3, 4); phase1b(5); phase1(7)
        phase1b(6); gate_group(1)
        phase1(8); phase1b(7); phase1(9)
        att_scores(1, 0, 1); phase1b(8)
        att_scores(1, 1, 2); phase1(10)
        att_scores(1, 2, 3); phase1b(9)
        att_scores(1, 3, 4); phase1(11)
        att_pv(0, 0, 2)
        att_scores(1, 4, 5); phase1b(10)
        att_pv(0, 2, 4)
        att_scores(1, 5, 6); gate_group(2)
        att_scores(1, 6, 7); phase1(12)
        att_scores(1, 7, 8); phase1b(11); phase1(13)
        att_tail(0)
        att_scores(2, 0, 1); phase1b(12)
        att_scores(2, 1, 2); phase1(14)
        att_scores(2, 2, 3); phase1b(13)
        att_scores(2, 3, 4); phase1(15)
        att_scores(2, 4, 6); phase1b(14)
        att_pv(1, 0, 2)
        att_scores(2, 6, 8); gate_group(3)
        att_pv(1, 2, 4)
        att_scores(2, 8, 10); phase1b(15)
        att_scores(2, 10, 12)
        att_tail(1)
        att_scores(3, 0, 4)
        att_pv(2, 0, 2)
        att_scores(3, 4, 8)
        att_pv(2, 2, 4)
        att_scores(3, 8, 13)
        att_scores(3, 13, 14)
        att_tail(2)
        att_pv(3, 0, 1)
        att_scores(3, 14, 15)
        att_scores(3, 15, 16)
        att_pv(3, 1, 2)
        att_pv(3, 2, 3)
        att_pv(3, 3, 4)
        att_tail(3)
        """
        env = locals()
        for line in sched.strip().split("\n"):
            for call in line.strip().split("; "):
                call = call.strip()
                if not call:
                    continue
                _mark(nc, call.replace(" ", ""))
                eval(call, {}, env)

        stack.close()

    nc.compile()
    return nc


def _host_prep(x, v1, Wq, Wk, Wv, Wproj, Wg, lamb):
    lamb = np.float32(lamb)
    half = HD // 2
    inv_freq = 1.0 / (ROPE_BASE ** (np.arange(0, HD, 2, dtype=np.float32) / HD))
    freqs = np.arange(T, dtype=np.float32)[:, None] * inv_freq[None, :]  # [T, 32]
    cosf = np.cos(freqs).astype(np.float32)
    sinf = np.sin(freqs).astype(np.float32)

    id16 = np.eye(128, dtype=np.float16)
    idbf = np.eye(128, dtype=ml_dtypes.bfloat16)
    # causal keep-mask in S^T layout: rows=k_local, cols=q_local; keep q >= k
    kk, qq = np.meshgrid(np.arange(128), np.arange(128), indexing="ij")
    msk16 = (qq >= kk).astype(np.float16)

    def quant_pair(w):
        w8 = (w * SW).astype(NP8)
        wr8 = (w * SW - w8.astype(np.float32)).astype(NP8)
        return w8, wr8

    def wtile(w8):
        # [D, OL] -> [128, (KD//2)*2*OL] with [p, (c,s,o)] = w8[(2c+s)*128+p, o]
        return np.ascontiguousarray(
            w8.reshape(KD // 2, 2, 128, OL).transpose(2, 0, 1, 3)
            .reshape(128, -1))

    WqT = np.ascontiguousarray(Wq.T).astype(np.float32)
    WkT = np.ascontiguousarray(Wk.T).astype(np.float32)
    WvT = (np.ascontiguousarray(Wv.T) * (1.0 - lamb)).astype(np.float32)

    in_maps = []
    for c in range(N_CORES):
        b, g = c // 4, c % 4
        hsl = slice(g * OL, (g + 1) * OL)

        # x8 / xr8 tiled [128, NT, 2*1024]
        xb = x[b].astype(np.float32) * SX     # [T, D]
        x8 = xb.astype(NP8)
        xr8 = (xb - x8.astype(np.float32)).astype(NP8)

        def xtile(x8_):
            # [T, D] -> [128p(feat), NT, KD*128] : [p, m, kd*128+t']
            # = x8_[128m+t', 128kd+p]
            a = x8_.reshape(NT, 128, KD, 128)        # [m, t', kd, p]
            return a.transpose(3, 0, 2, 1).reshape(128, NT, KD * 128)

        xx8 = np.concatenate([xtile(x8), xtile(xr8)], axis=2)  # [128, NT, 2048]

        # vcs: [128, NT, 384] f16 : lam*v1 (256) | cos pair (64) | sin pair (64)
        v1c = (v1[b, :, g * HL:(g + 1) * HL, :].astype(np.float32) * lamb)
        v1t = v1c.reshape(NT, 128, OL).transpose(1, 0, 2)      # [t', m, 256]
        cs = np.empty((128, NT, 2, 2, 32), dtype=np.float32)
        cosr = cosf.reshape(NT, 128, 32).transpose(1, 0, 2)    # [t', m, 32]
        sinr = sinf.reshape(NT, 128, 32).transpose(1, 0, 2)
        cs[:, :, 0, 0] = 0.1 * cosr
        cs[:, :, 0, 1] = 0.1 * sinr
        cs[:, :, 1, 0] = cosr
        cs[:, :, 1, 1] = sinr
        # kernel reads ctv = [256:320] = [a, 32] cos ; stv = [320:384] sin
        vcs = np.empty((128, NT, 384), dtype=np.float16)
        vcs[:, :, 0:256] = v1t
        vcs[:, :, 256:288] = cs[:, :, 0, 0]
        vcs[:, :, 288:320] = cs[:, :, 1, 0]
        vcs[:, :, 320:352] = cs[:, :, 0, 1]
        vcs[:, :, 352:384] = cs[:, :, 1, 1]

        q8, qr8 = quant_pair(WqT[:, hsl])
        k8, kr8 = quant_pair(WkT[:, hsl])
        v8, vr8 = quant_pair(WvT[:, hsl])

        in_maps.append({
            "xx8": np.ascontiguousarray(xx8),
            "vcs": np.ascontiguousarray(vcs),
            "wq8": wtile(q8), "wqr8": wtile(qr8),
            "wk8": wtile(k8), "wkr8": wtile(kr8),
            "wv8": wtile(v8), "wvr8": wtile(vr8),
            "wp": np.ascontiguousarray(
                Wproj[:, hsl].T.reshape(2, 128, D).transpose(1, 0, 2)
            ).astype(ml_dtypes.bfloat16),
            "wg": np.ascontiguousarray(Wg[g * HL:(g + 1) * HL, :].T)
                 .astype(np.float16),
            "xg": np.ascontiguousarray(x[b, :, :12].T).astype(np.float16),
            "id16": id16, "idbf": idbf, "msk16": msk16,
        })
    return in_maps


def kernel(x, v1, Wq, Wk, Wv, Wproj, Wg, lamb, **run_kwargs):
    x = np.asarray(x, dtype=np.float32)
    v1 = np.asarray(v1, dtype=np.float32)
    if "nc" not in _CACHE:
        _CACHE["nc"] = build_program()
    nc = _CACHE["nc"]
    in_maps = _host_prep(x, np.asarray(v1), np.asarray(Wq), np.asarray(Wk),
                         np.asarray(Wv), np.asarray(Wproj), np.asarray(Wg),
                         np.float32(lamb))
    res = run_bass_kernel_spmd(nc, in_maps, core_ids=list(range(N_CORES)),
                               **run_kwargs)
    _CACHE["last_results"] = res
    out = np.zeros((B, T, D), dtype=np.float32)
    for c in range(N_CORES):
        b = c // 4
        out[b] += res.results[c]["outT"].T.astype(np.float32)
    return out


# revision 115
# speedup vs baseline: 1.0053x; 1.0053x over previous
"""Causal self-attention (value-residual + QK RMSNorm + RoPE + sigmoid gate)
Trainium2 Bass kernel, sharded over 8 NeuronCores.

Sharding: core c handles batch b = c // 4 and the 4 heads [4*(c%4), 4*(c%4)+4).
Each core computes its heads' QKV, attention and gating, then a partial
c_proj contribution out_partial^T = Wproj[:, islice] @ y_gated^T  [1024, 2048]
in f16; host sums the 4 partials per batch in f32 and transposes back.

Key optimizations over the v1 kernel:
 - QKV projections in fp8 (e4m3) DoubleRow matmuls with 3-term residual
   correction: q = x8@W8 + x8@Wr8 + xr8@W8 (likewise k, v).  Host prepares
   x8/xr8 and W8/Wr8; PE cost drops to 0.75x of the f16 path at full
   precision recovery (error ~ quantization^2).
 - Phase interleaving: attention for q-chunk qc only needs the first
   4*(qc+1) token tiles of phase 1, so phase-1 m-groups are emitted
   between attention chunks and all five engines stay busy.
 - Engine rebalance: PSUM->SBUF copies and v-mix on Pool (gpsimd), RMS
   stats/RoPE/masks/y-scale on DVE, exp-only on ACT, output DMAs issued
   from the ACT queue, input DMAs from SP.
 - y path in bf16 (halves y-transpose cost), c_proj keeps bf16 weights.
 - One fused input DMA per token tile (x8|xr8) plus one for (v1|cos|sin).
 - 4-head y accumulator in one PSUM bank with batched normalization
   (one reciprocal + one gate*rec + one apply per j instead of four).
 - Paired PSUM drains: both transpose halves share one psum tile and are
   drained by a single 256-wide copy; cproj's two half-rounds accumulate
   into one full-bank psum tile with a single 512-wide drain; the four
   gate matmuls share one psum tile (pending-zero writes) with one drain.
 - All cproj/ytp drains on DVE (ACT runs exps only in the steady state);
   early qk-transpose drains (m<12) ride the then-idle ACT.
 - f16 gate path and f16 output staging/DMA (host sums partials in f32).
"""

import sys

sys.path.insert(0, "/opt/trn_rl_repo")

import math

import numpy as np
import ml_dtypes

import concourse.bass as bass
import concourse.mybir as mybir
import concourse.tile as tile
from concourse.bass_utils import run_bass_kernel_spmd
from concourse import bacc


# Force Exp and Ln to resolve to the combined natural_log_exp_and_others set:
# the greedy table-load pass otherwise alternates exp_and_others/natural_log,
# inserting a table load per switch. Entry positions are preserved so
# act_func_set_id indices stay valid.
_orig_gat = bacc.get_activation_tables


def _gat_combined(arch):
    out = {}
    for name, fns in _orig_gat(arch).items():
        if name != "natural_log_exp_and_others":
            fns = {f for f in fns
                   if str(f).split(".")[-1] not in ("Exp", "Ln")}
        out[name] = fns
    return out


bacc.get_activation_tables = _gat_combined

F32 = mybir.dt.float32
F32R = mybir.dt.float32r
F16 = mybir.dt.float16
BF16 = mybir.dt.bfloat16
F8 = mybir.dt.float8e4
AF = mybir.ActivationFunctionType
OP = mybir.AluOpType
DR = mybir.MatmulPerfMode.DoubleRow

B, T, D, H, HD = 2, 2048, 1024, 16, 64
HL = 4            # heads per core
OL = HL * HD      # 256 local qkv width
NT = T // 128     # 16 t-tiles
KD = D // 128     # 8 contraction tiles
NQC = T // 512    # 4 q chunks
EPS = 1.1920929e-7
ATTN_SCALE = 0.1
ROPE_BASE = 10000.0
N_CORES = 8
NP8 = ml_dtypes.float8_e4m3
SX = 32.0         # fp8 scale for x8/xr8 (e4m3 max is 240; |x| < 6)
SW = 64.0         # fp8 scale for W8/Wr8 (|W| <= 1/32)
DEQ = 1.0 / (SX * SW)

_CACHE = {}
PHASE_MARKS = []   # (label, first_instruction_number) in emission order


def _mark(nc, label):
    n = nc._state.get_next_instruction_name()  # consumes one name
    PHASE_MARKS.append((label, int(n.split("-")[1])))


def r32(ap):
    return ap.bitcast(F32R)


def build_program():
    nc = bacc.Bacc("TRN2", target_bir_lowering=False, debug=False, num_devices=1)

    # host-tiled inputs (layouts chosen for single fat DMAs, no elem penalty)
    xx8 = nc.dram_tensor("xx8", [128, NT, 2 * KD * 128], F8,
                         kind="ExternalInput").ap()          # x8 | xr8
    vcs = nc.dram_tensor("vcs", [128, NT, 384], F16,
                         kind="ExternalInput").ap()          # lam*v1 | cos | sin
    WSZ = (KD // 2) * 2 * OL
    wq8 = nc.dram_tensor("wq8", [128, WSZ], F8, kind="ExternalInput").ap()
    wk8 = nc.dram_tensor("wk8", [128, WSZ], F8, kind="ExternalInput").ap()
    wv8 = nc.dram_tensor("wv8", [128, WSZ], F8, kind="ExternalInput").ap()
    wqr8 = nc.dram_tensor("wqr8", [128, WSZ], F8, kind="ExternalInput").ap()
    wkr8 = nc.dram_tensor("wkr8", [128, WSZ], F8, kind="ExternalInput").ap()
    wvr8 = nc.dram_tensor("wvr8", [128, WSZ], F8, kind="ExternalInput").ap()
    wp = nc.dram_tensor("wp", [128, 2, D], BF16, kind="ExternalInput").ap()
    wg = nc.dram_tensor("wg", [12, HL], F16, kind="ExternalInput").ap()
    xg = nc.dram_tensor("xg", [12, T], F16, kind="ExternalInput").ap()
    id16 = nc.dram_tensor("id16", [128, 128], F16, kind="ExternalInput").ap()
    idbf = nc.dram_tensor("idbf", [128, 128], BF16, kind="ExternalInput").ap()
    msk16 = nc.dram_tensor("msk16", [128, 128], F16, kind="ExternalInput").ap()
    outT = nc.dram_tensor("outT", [D, T], F16, kind="ExternalOutput").ap()

    with tile.TileContext(nc) as tc:
        import contextlib
        stack = contextlib.ExitStack()
        pers = stack.enter_context(tc.tile_pool(name="pers", bufs=1))
        xp = stack.enter_context(tc.tile_pool(name="xp", bufs=5))
        sp = stack.enter_context(tc.tile_pool(name="sp", bufs=4))
        pp = stack.enter_context(tc.tile_pool(name="pp", bufs=50))
        op_ = stack.enter_context(tc.tile_pool(name="op", bufs=10))
        yp = stack.enter_context(tc.tile_pool(name="yp", bufs=2))
        # PSUM budget is 8 banks of 2KB; every ring slot is bank-rounded.
        # pss: 2 slots x 2 banks.  psa: tag "ph1" (qkv/transpose/gate psums)
        # and tag "attn" (y/ytranspose/cproj psums), 2 bank-slots each.
        pss = stack.enter_context(tc.tile_pool(name="pss", bufs=2, space="PSUM"))
        psa = stack.enter_context(tc.tile_pool(name="psa", bufs=2, space="PSUM"))

        # ---- persistent tiles / weight DMAs (issue order = need order) ----
        def ptile(shape, dtype, name):
            return pers.tile(shape, dtype, name=name, tag=name, bufs=1)

        def wview(t):
            return t.rearrange("p (c s o) -> p c s o", s=2, o=OL)

        wq8_f = ptile([128, WSZ], F8, "wq8_sb")
        wqr8_f = ptile([128, WSZ], F8, "wqr8_sb")
        wk8_f = ptile([128, WSZ], F8, "wk8_sb")
        wkr8_f = ptile([128, WSZ], F8, "wkr8_sb")
        wv8_f = ptile([128, WSZ], F8, "wv8_sb")
        wvr8_f = ptile([128, WSZ], F8, "wvr8_sb")
        nc.sync.dma_start(out=wq8_f, in_=wq8)
        nc.sync.dma_start(out=wqr8_f, in_=wqr8)

        def kv_dma_hook(nm):
            # called during phase1(0) emission, before the k / v matmuls
            if nm == "k":
                nc.sync.dma_start(out=wk8_f, in_=wk8)
                nc.sync.dma_start(out=wkr8_f, in_=wkr8)
            elif nm == "v":
                nc.sync.dma_start(out=wv8_f, in_=wv8)
                nc.sync.dma_start(out=wvr8_f, in_=wvr8)
        wq8_sb, wqr8_sb = wview(wq8_f), wview(wqr8_f)
        wk8_sb, wkr8_sb = wview(wk8_f), wview(wkr8_f)
        wv8_sb, wvr8_sb = wview(wv8_f), wview(wvr8_f)
        id16_sb = ptile([128, 128], F16, "id16_sb")
        idbf_sb = ptile([128, 128], BF16, "idbf_sb")
        msk_sb = ptile([128, 128], F16, "msk_sb")
        xg_sb = ptile([12, T], F16, "xg_sb")
        wg_sb = ptile([12, HL], F16, "wg_sb")
        nc.sync.dma_start(out=id16_sb, in_=id16)
        wp_sb = ptile([128, 2, D], BF16, "wp_sb")

        def late_dmas():
            nc.sync.dma_start(out=msk_sb, in_=msk16)
            nc.sync.dma_start(out=xg_sb, in_=xg)
            nc.sync.dma_start(out=wg_sb, in_=wg)
            nc.sync.dma_start(out=idbf_sb, in_=idbf)
            nc.sync.dma_start(out=wp_sb, in_=wp)

        eps_sb = ptile([128, 1], F32, "eps_sb")
        nc.vector.memset(eps_sb, EPS)

        v_m = ptile([128, NT, HL, 66], F16, "v_m")
        nc.gpsimd.memset(v_m[:, :, :, 64:66], 1.0)
        gate_all = ptile([128, NT, HL], F32, "gate_all")
        kTt = ptile([128, NT, 2, 128], F16, "kTt")
        qTc = ptile([128, NQC, 2, 512], F16, "qTc")
        ysb = ptile([128, NT, 2, 128], BF16, "ysb")

        rots = [None] * NT

        # ---- phase 1a for one token tile m -------------------------------
        def phase1(m):
            xt = xp.tile([128, 2 * KD * 128], F8, name=f"xx{m}", tag="xx")
            nc.sync.dma_start(out=xt, in_=xx8[:, m])
            vt = xp.tile([128, 384], F16, name=f"vcs{m}", tag="vcs")
            nc.sync.dma_start(out=vt, in_=vcs[:, m])
            x8v = xt[:, 0:1024].rearrange("p (c s t) -> p c s t", s=2, t=128)
            xr8v = xt[:, 1024:2048].rearrange("p (c s t) -> p c s t", s=2, t=128)
            v1t = vt[:, 0:256].rearrange("p (h d) -> p h d", h=HL)
            ctv = (vt[:, 256:320].rearrange("p (a f) -> p a f", a=2)
                   .unsqueeze(2).broadcast_to([128, 2, HL, 32]))
            stv = (vt[:, 320:384].rearrange("p (a f) -> p a f", a=2)
                   .unsqueeze(2).broadcast_to([128, 2, HL, 32]))

            qk_ps = psa.tile([128, 2, OL], F32, name=f"qk_ps{m}", tag="ph1")
            pses = []
            for w8, wr8, nm in ((wq8_sb, wqr8_sb, "q"), (wk8_sb, wkr8_sb, "k"),
                                (wv8_sb, wvr8_sb, "v")):
                if m == 0 and nm != "q":
                    kv_dma_hook(nm)
                if nm == "q":
                    ps = qk_ps[:, 0]
                elif nm == "k":
                    ps = qk_ps[:, 1]
                else:
                    ps = psa.tile([128, OL], F32, name=f"v_ps{m}", tag="ph1")
                nc.tensor.matmul(ps, x8v[:, 0], w8[:, 0], start=True,
                                 stop=False, perf_mode=DR)
                for c in range(1, 4):
                    nc.tensor.matmul(ps, x8v[:, c], w8[:, c], start=False,
                                     stop=False, perf_mode=DR)
                for c in range(4):
                    nc.tensor.matmul(ps, x8v[:, c], wr8[:, c], start=False,
                                     stop=False, perf_mode=DR)
                for c in range(4):
                    nc.tensor.matmul(ps, xr8v[:, c], w8[:, c], start=False,
                                     stop=(c == 3), perf_mode=DR)
                pses.append(ps)
            q_ps, k_ps, v_ps = pses

            # PSUM carries x8@W8-style products scaled by SX*SW = 2048.
            # DVE dequant-copies drain PSUM (Pool cannot touch PSUM, and a
            # DVE op may read at most one PSUM operand).
            qks = sp.tile([128, 2, HL, HD], F16, name=f"qks{m}", tag="qks")
            nc.vector.tensor_scalar_mul(
                qks, qk_ps.rearrange("t a (h d) -> t a h d", h=HL), DEQ)
            nc.vector.scalar_tensor_tensor(
                out=v_m[:, m, :, 0:64],
                in0=v_ps.rearrange("t (h d) -> t h d", h=HL),
                scalar=DEQ, in1=v1t, op0=OP.mult, op1=OP.add)

            qkf = qks.rearrange("t a h d -> t (a h d)")
            scr = sp.tile([128, 2 * OL], F16, name=f"scr{m}", tag="scr")
            nc.vector.tensor_mul(scr, qkf, qkf)
            ss = sp.tile([128, 8], F16, name=f"ss{m}", tag="ss")
            with nc.allow_low_precision(reason="64-term f16 sum feeds rsqrt"):
                nc.vector.tensor_reduce(
                    out=ss, in_=scr.rearrange("t (g d) -> t g d", d=HD),
                    axis=mybir.AxisListType.X, op=OP.add)
            lnv = sp.tile([128, 8], F32, name=f"lnv{m}", tag="lnv")
            nc.scalar.activation(lnv, ss, AF.Ln, scale=1.0 / HD, bias=eps_sb)
            rs = sp.tile([128, 2, HL], F16, name=f"rs{m}", tag="rs")
            # rs = rsqrt(mean q^2); 0.1 for q is folded into host cos/sin.
            nc.scalar.activation(rs.rearrange("t a h -> t (a h)"), lnv,
                                 AF.Exp, scale=-0.5)
            rs2 = sp.tile([128, 2, HL, 2], F16, name=f"rs2{m}", tag="rs2")
            nc.gpsimd.tensor_copy(
                rs2, rs.unsqueeze(3).broadcast_to([128, 2, HL, 2]))
            qk = sp.tile([128, 2, HL, HD], F16, name=f"qk{m}", tag="qk")
            nc.vector.tensor_mul(
                qk.rearrange("t a h (f e) -> t a h f e", e=2), qks.rearrange(
                    "t a h (f e) -> t a h f e", e=2),
                rs2.unsqueeze(3).broadcast_to([128, 2, HL, 32, 2]))

            a_, b_ = qk[:, :, :, 0:32], qk[:, :, :, 32:64]
            rot = sp.tile([128, 2, HL, HD], F16, name=f"rot{m}", tag="rot",
                          bufs=3)
            r1, r2 = rot[:, :, :, 0:32], rot[:, :, :, 32:64]
            tmp = sp.tile([128, 2, HL, 32], F16, name=f"tmp{m}", tag="tmp")
            tmp2 = sp.tile([128, 2, HL, 32], F16, name=f"tmp2{m}", tag="tmp2")
            nc.vector.tensor_mul(tmp, b_, stv)
            nc.vector.tensor_mul(r1, a_, ctv)
            nc.vector.tensor_add(r1, r1, tmp)
            nc.gpsimd.tensor_mul(tmp2, a_, stv)
            nc.vector.tensor_mul(r2, b_, ctv)
            nc.vector.tensor_sub(r2, r2, tmp2)

            rots[m] = rot

        # ---- phase 1b: transposes for tile m (emitted later so they do
        # not block the next tile's QKV matmuls in the in-order PE queue) --
        def phase1b(m):
            # early tiles drain their transposes on the then-idle ACT;
            # both halves share one psum tile -> ONE 256-wide drain copy
            cp = nc.scalar.copy if m < 12 else nc.vector.tensor_copy
            rotf = rots[m].rearrange("t a h d -> t (a h d)")  # q 0-255 k 256-511
            tps = psa.tile([128, 2, 128], F16, name=f"tpq{m}", tag="attn")
            for half in range(2):
                nc.tensor.transpose(tps[:, half],
                                    rotf[:, 128 * half:128 * half + 128],
                                    id16_sb)
            cp(qTc[:, m // 4, :, 128 * (m % 4):128 * (m % 4) + 128], tps)
            tpk = psa.tile([128, 2, 128], F16, name=f"tpk{m}", tag="attn")
            for half in range(2):
                nc.tensor.transpose(tpk[:, half],
                                    rotf[:, 256 + 128 * half:384 + 128 * half],
                                    id16_sb)
            cp(kTt[:, m], tpk)

        # ---- gate for a group of 4 token tiles ---------------------------
        def gate_group(g):
            gg = sp.tile([128, 4, HL], F32, name=f"gg{g}", tag="gg")
            g_ps = psa.tile([128, 4, HL], F32, name=f"g_ps{g}", tag="attn")
            # one zero-region group; starts mark the region pending-zero so
            # each disjoint [*, mi] write replaces rather than accumulates
            for mi in range(4):
                m = 4 * g + mi
                nc.tensor.matmul(g_ps[:, mi], xg_sb[:, 128 * m:128 * m + 128],
                                 wg_sb, start=(mi == 0), stop=(mi == 3))
            nc.vector.tensor_copy(gg, g_ps)
            gge = sp.tile([128, 4, HL], F32, name=f"gge{g}", tag="gge")
            nc.scalar.activation(gge.rearrange("t a h -> t (a h)"),
                                 gg.rearrange("t a h -> t (a h)"),
                                 AF.Exp, scale=-1.0)
            nc.gpsimd.tensor_scalar_add(gge, gge, 1.0)
            nc.vector.reciprocal(
                gate_all[:, 4 * g:4 * g + 4].rearrange("t a h -> t (a h)"),
                gge.rearrange("t a h -> t (a h)"))

        # ---- attention for one q chunk, split into schedulable blocks ----
        pts_all = [[[None] * NT for _ in range(2)] for _ in range(NQC)]

        def att_scores(qc, i0=0, i1=None):
            pts = pts_all[qc]
            for i in range(i0, 4 * qc + 4 if i1 is None else i1):
                ql0 = max(0, 128 * (i - 4 * qc))
                ncols = 512 - ql0
                for p in range(2):
                    s_ps = pss.tile([128, 2, 512], F32, name=f"s{p}_{qc}_{i}",
                                    tag="pss")
                    for r in range(2):
                        prt = slice(64 * r, 64 * r + 64)
                        nc.tensor.matmul(
                            s_ps[:, r, 0:ncols],
                            kTt[prt, i, p],
                            qTc[prt, qc, p, ql0:512],
                            start=True, stop=True,
                            tile_position=(64 * r, 0))
                    pt = pp.tile([128, 2, 512], F16, name=f"pt{p}_{qc}_{i}",
                                 tag="pt")
                    nc.scalar.activation(pt[:, :, ql0:512], s_ps[:, :, 0:ncols],
                                         AF.Exp)
                    if i >= 4 * qc:
                        mb = msk_sb.unsqueeze(1).broadcast_to([128, 2, 128])
                        mul = nc.vector.tensor_mul if qc >= 2 \
                            else nc.gpsimd.tensor_mul
                        mul(pt[:, :, ql0:ql0 + 128],
                            pt[:, :, ql0:ql0 + 128], mb)
                    pts[p][i] = pt

        def att_pv(qc, js0=0, js1=4):
            # 4-head y accumulator in ONE psum bank; batched normalize:
            # rec4 = 1/rowsum (DVE), rg = rec*gate (Pool), one apply (DVE).
            pts = pts_all[qc]
            for js in range(js0, js1):
                j = 4 * qc + js
                y_ps = psa.tile([128, HL, 66], F32, name=f"y_{j}", tag="attn")
                for p in range(2):
                    for r in range(2):
                        h = 2 * p + r
                        for i in range(j + 1):
                            nc.tensor.matmul(
                                y_ps[:, h],
                                pts[p][i][:, r, 128 * js:128 * js + 128],
                                v_m[:, i, h, 0:66],
                                start=(i == 0), stop=(i == j))
                rec4 = sp.tile([128, HL], F32, name=f"rec{j}", tag="rec")
                nc.vector.reciprocal(
                    rec4, y_ps[:, :, 64:65].rearrange("t h o -> t (h o)"))
                rg = sp.tile([128, HL], F32, name=f"rg{j}", tag="rg")
                rgmul = nc.vector.tensor_mul if qc == 3 \
                    else nc.gpsimd.tensor_mul
                rgmul(rg, rec4, gate_all[:, j])
                ysb_j = (ysb[:, j].rearrange("t p d -> t (p d)")
                         .rearrange("t (h e) -> t h e", h=HL))
                nc.vector.tensor_mul(
                    ysb_j, y_ps[:, :, 0:64],
                    rg.unsqueeze(2).broadcast_to([128, HL, 64]))

        yTq_c = [None] * NQC
        ots = [None] * 8

        def ensure_yT(qc):
            if yTq_c[qc] is None:
                yTq_c[qc] = yp.tile([128, 2, 512], BF16, name=f"yT{qc}",
                                    tag="yT")
            return yTq_c[qc]

        def att_ytp(qc, js):
            # PE-transpose ysb[j] into yTq columns (+DVE drains)
            yTq = ensure_yT(qc)
            j = 4 * qc + js
            for half in range(2):
                ytp = psa.tile([128, 128], BF16, name=f"ty{j}{half}",
                               tag="attn")
                nc.tensor.transpose(ytp, ysb[:, j, half], idbf_sb)
                nc.vector.tensor_copy(yTq[:, half, 128 * js:128 * js + 128],
                                      ytp)

        def att_pv_j(qc, js, i0, i1, stop):
            # partial PV chain for tier js: i in [i0, i1); psum group left
            # open unless stop. Caller finishes with att_pv_fin.
            pts = pts_all[qc]
            j = 4 * qc + js
            if i0 == 0:
                pv_ps[(qc, js)] = psa.tile([128, HL, 66], F32,
                                           name=f"y_{j}", tag="ph1")
            y_ps = pv_ps[(qc, js)]
            for p in range(2):
                for r in range(2):
                    h = 2 * p + r
                    for i in range(i0, i1):
                        nc.tensor.matmul(
                            y_ps[:, h],
                            pts[p][i][:, r, 128 * js:128 * js + 128],
                            v_m[:, i, h, 0:66],
                            start=(i == 0), stop=(stop and i == i1 - 1))

        def att_pv_fin(qc, js):
            j = 4 * qc + js
            y_ps = pv_ps[(qc, js)]
            rec4 = sp.tile([128, HL], F32, name=f"rec{j}", tag="rec")
            nc.vector.reciprocal(
                rec4, y_ps[:, :, 64:65].rearrange("t h o -> t (h o)"))
            rg = sp.tile([128, HL], F32, name=f"rg{j}", tag="rg")
            nc.gpsimd.tensor_mul(rg, rec4, gate_all[:, j])
            ysb_j = (ysb[:, j].rearrange("t p d -> t (p d)")
                     .rearrange("t (h e) -> t h e", h=HL))
            nc.vector.tensor_mul(
                ysb_j, y_ps[:, :, 0:64],
                rg.unsqueeze(2).broadcast_to([128, HL, 64]))

        pv_ps = {}

        def att_cproj(qc, tc2, dma):
            yTq = ensure_yT(qc)
            lsl = slice(256 * tc2, 256 * tc2 + 256)
            tsl = slice(512 * qc, 512 * qc + 512)
            for oc in range(8):
                osl = slice(128 * oc, 128 * oc + 128)
                if tc2 == 0:
                    ots[oc] = op_.tile([128, 512], F16,
                                       name=f"ot{oc}_{qc}", tag="ot")
                pr = psa.tile([128, 256], F32, name=f"pr{oc}_{qc}_{tc2}",
                              tag="attn")
                for a in range(2):
                    nc.tensor.matmul(pr, wp_sb[:, a, osl],
                                     yTq[:, a, lsl],
                                     start=(a == 0), stop=(a == 1))
                nc.vector.tensor_copy(ots[oc][:, lsl], pr)
                if dma:
                    nc.sync.dma_start(out=outT[osl, tsl], in_=ots[oc])

        def att_tail(qc, part=None):
            # tail drains ride ACT in round 0/1 (ACT slack); DVE otherwise.
            # qc3 (post-last-exp, ACT idle): alternate copies ACT/DVE so the
            # final drain runs on both engines in parallel.
            on_act = False
            ot_copy = nc.scalar.copy if on_act else nc.vector.tensor_copy
            ytp_copy = nc.vector.tensor_copy
            if yTq_c[qc] is None:
                yTq_c[qc] = yp.tile([128, 2, 512], BF16, name=f"yT{qc}",
                                    tag="yT")
            yTq = yTq_c[qc]
            parts = (0, 1) if part is None else (part,)
            for tc2 in parts:
                for js in (2 * tc2, 2 * tc2 + 1):
                    j = 4 * qc + js
                    ytp = psa.tile([128, 2, 128], BF16, name=f"ty{j}",
                                   tag="attn")
                    for half in range(2):
                        nc.tensor.transpose(ytp[:, half], ysb[:, j, half],
                                            idbf_sb)
                    ytp_copy(yTq[:, :, 128 * js:128 * js + 128], ytp)
            if part is None:
                tsl = slice(512 * qc, 512 * qc + 512)
                for oc in range(8):
                    osl = slice(128 * oc, 128 * oc + 128)
                    ot = op_.tile([128, 512], F16, name=f"ot{oc}_{qc}",
                                  tag="ot")
                    # qc3: odd oc borrow the idle ph1 psum ring -> 4
                    # cproj rounds in flight instead of 2 at the very end
                    ptag = "ph1" if (qc == 3 and oc % 2 == 1) else "attn"
                    pr = psa.tile([128, 512], F32, name=f"pr{oc}_{qc}",
                                  tag=ptag)
                    for tc2 in range(2):
                        lsl = slice(256 * tc2, 256 * tc2 + 256)
                        for a in range(2):
                            nc.tensor.matmul(pr[:, lsl], wp_sb[:, a, osl],
                                             yTq[:, a, lsl],
                                             start=(a == 0), stop=(a == 1))
                    cp = (nc.scalar.copy if (qc == 3 and oc % 2 == 1)
                          else ot_copy)
                    cp(ot, pr)
                    nc.sync.dma_start(out=outT[osl, tsl], in_=ot)
            else:
                # half-tail: fill ot[:, half]; the tc2=1 half emits the DMA
                tc2 = part
                lsl = slice(256 * tc2, 256 * tc2 + 256)
                tsl = slice(512 * qc, 512 * qc + 512)
                for oc in range(8):
                    osl = slice(128 * oc, 128 * oc + 128)
                    if tc2 == 0:
                        ots[oc] = op_.tile([128, 512], F16,
                                           name=f"ot{oc}_{qc}", tag="ot")
                    pr = psa.tile([128, 256], F32, name=f"pr{oc}_{qc}_{tc2}",
                                  tag="attn")
                    for a in range(2):
                        nc.tensor.matmul(pr, wp_sb[:, a, osl],
                                         yTq[:, a, lsl],
                                         start=(a == 0), stop=(a == 1))
                    ot_copy(ots[oc][:, lsl], pr)
                    if tc2 == 1:
                        nc.sync.dma_start(out=outT[osl, tsl], in_=ots[oc])

        # ---- schedule (software-pipelined; per-engine queues are in-order
        # so later-dependent PE work is always emitted after independent
        # PE work that can run now) ----------------------------------------
        # ---- schedule: phase-1 tiles staggered so two QKV tile-loads run
        # ahead of the chain, and the next group's first tiles are issued
        # before each score round to keep PE fed during chain tails. -------
        sched = """
        phase1(0)
        phase1(1)
        late_dmas()
        phase1b(0); phase1(2)
        phase1b(1); phase1(3)
        phase1b(2); gate_group(0)
        phase1(4); phase1b(3)
        att_scores(0, 0, 1); phase1(5)
        att_scores(0, 1, 2); phase1b(4)
        att_scores(0, 2, 3); phase1(6)
        att_scores(0, 3, 4); phase1b(5); phase1(7)
        phase1b(6); gate_group(1)
        phase1(8); phase1b(7); phase1(9)
        att_scores(1, 0, 1); phase1b(8)
        att_scores(1, 1, 2); phase1(10)
        att_scores(1, 2, 3); phase1b(9)
        att_scores(1, 3, 4); phase1(11)
        att_pv(0, 0, 2)
        att_scores(1, 4, 5); phase1b(10)
        att_pv(0, 2, 4)
        att_scores(1, 5, 6); gate_group(2)
        att_scores(1, 6, 7); phase1(12)
        att_scores(1, 7, 8); phase1b(11); phase1(13)
        att_tail(0)
        att_scores(2, 0, 1); phase1b(12)
        att_scores(2, 1, 2); phase1(14)
        att_scores(2, 2, 3); phase1b(13)
        att_scores(2, 3, 4); phase1(15)
        att_scores(2, 4, 6); phase1b(14)
        att_pv(1, 0, 2)
        att_scores(2, 6, 8); gate_group(3)
        att_pv(1, 2, 4)
        att_scores(2, 8, 10); phase1b(15)
        att_scores(2, 10, 12)
        att_tail(1)
        att_scores(3, 0, 4)
        att_pv(2, 0, 2)
        att_scores(3, 4, 8)
        att_pv(2, 2, 4)
        att_scores(3, 8, 13)
        att_scores(3, 13, 14)
        att_tail(2)
        att_pv(3, 0, 1)
        att_scores(3, 14, 15)
        att_scores(3, 15, 16)
        att_pv(3, 1, 2)
        att_pv(3, 2, 3)
        att_pv(3, 3, 4)
        att_tail(3)
        """
        env = locals()
        for line in sched.strip().split("\n"):
            for call in line.strip().split("; "):
                call = call.strip()
                if not call:
                    continue
                _mark(nc, call.replace(" ", ""))
                eval(call, {}, env)

        stack.close()

    nc.compile()
    return nc


def _host_prep(x, v1, Wq, Wk, Wv, Wproj, Wg, lamb):
    lamb = np.float32(lamb)
    half = HD // 2
    inv_freq = 1.0 / (ROPE_BASE ** (np.arange(0, HD, 2, dtype=np.float32) / HD))
    freqs = np.arange(T, dtype=np.float32)[:, None] * inv_freq[None, :]  # [T, 32]
    cosf = np.cos(freqs).astype(np.float32)
    sinf = np.sin(freqs).astype(np.float32)

    id16 = np.eye(128, dtype=np.float16)
    idbf = np.eye(128, dtype=ml_dtypes.bfloat16)
    # causal keep-mask in S^T layout: rows=k_local, cols=q_local; keep q >= k
    kk, qq = np.meshgrid(np.arange(128), np.arange(128), indexing="ij")
    msk16 = (qq >= kk).astype(np.float16)

    def quant_pair(w):
        w8 = (w * SW).astype(NP8)
        wr8 = (w * SW - w8.astype(np.float32)).astype(NP8)
        return w8, wr8

    def wtile(w8):
        # [D, OL] -> [128, (KD//2)*2*OL] with [p, (c,s,o)] = w8[(2c+s)*128+p, o]
        return np.ascontiguousarray(
            w8.reshape(KD // 2, 2, 128, OL).transpose(2, 0, 1, 3)
            .reshape(128, -1))

    WqT = np.ascontiguousarray(Wq.T).astype(np.float32)
    WkT = np.ascontiguousarray(Wk.T).astype(np.float32)
    WvT = (np.ascontiguousarray(Wv.T) * (1.0 - lamb)).astype(np.float32)

    in_maps = []
    for c in range(N_CORES):
        b, g = c // 4, c % 4
        hsl = slice(g * OL, (g + 1) * OL)

        # x8 / xr8 tiled [128, NT, 2*1024]
        xb = x[b].astype(np.float32) * SX     # [T, D]
        x8 = xb.astype(NP8)
        xr8 = (xb - x8.astype(np.float32)).astype(NP8)

        def xtile(x8_):
            # [T, D] -> [128p(feat), NT, KD*128] : [p, m, kd*128+t']
            # = x8_[128m+t', 128kd+p]
            a = x8_.reshape(NT, 128, KD, 128)        # [m, t', kd, p]
            return a.transpose(3, 0, 2, 1).reshape(128, NT, KD * 128)

        xx8 = np.concatenate([xtile(x8), xtile(xr8)], axis=2)  # [128, NT, 2048]

        # vcs: [128, NT, 384] f16 : lam*v1 (256) | cos pair (64) | sin pair (64)
        v1c = (v1[b, :, g * HL:(g + 1) * HL, :].astype(np.float32) * lamb)
        v1t = v1c.reshape(NT, 128, OL).transpose(1, 0, 2)      # [t', m, 256]
        cs = np.empty((128, NT, 2, 2, 32), dtype=np.float32)
        cosr = cosf.reshape(NT, 128, 32).transpose(1, 0, 2)    # [t', m, 32]
        sinr = sinf.reshape(NT, 128, 32).transpose(1, 0, 2)
        cs[:, :, 0, 0] = 0.1 * cosr
        cs[:, :, 0, 1] = 0.1 * sinr
        cs[:, :, 1, 0] = cosr
        cs[:, :, 1, 1] = sinr
        # kernel reads ctv = [256:320] = [a, 32] cos ; stv = [320:384] sin
        vcs = np.empty((128, NT, 384), dtype=np.float16)
        vcs[:, :, 0:256] = v1t
        vcs[:, :, 256:288] = cs[:, :, 0, 0]
        vcs[:, :, 288:320] = cs[:, :, 1, 0]
        vcs[:, :, 320:352] = cs[:, :, 0, 1]
        vcs[:, :, 352:384] = cs[:, :, 1, 1]

        q8, qr8 = quant_pair(WqT[:, hsl])
        k8, kr8 = quant_pair(WkT[:, hsl])
        v8, vr8 = quant_pair(WvT[:, hsl])

        in_maps.append({
            "xx8": np.ascontiguousarray(xx8),
            "vcs": np.ascontiguousarray(vcs),
            "wq8": wtile(q8), "wqr8": wtile(qr8),
            "wk8": wtile(k8), "wkr8": wtile(kr8),
            "wv8": wtile(v8), "wvr8": wtile(vr8),
            "wp": np.ascontiguousarray(
                Wproj[:, hsl].T.reshape(2, 128, D).transpose(1, 0, 2)
            ).astype(ml_dtypes.bfloat16),
            "wg": np.ascontiguousarray(Wg[g * HL:(g + 1) * HL, :].T)
                 .astype(np.float16),
            "xg": np.ascontiguousarray(x[b, :, :12].T).astype(np.float16),
            "id16": id16, "idbf": idbf, "msk16": msk16,
        })
    return in_maps


def kernel(x, v1, Wq, Wk, Wv, Wproj, Wg, lamb, **run_kwargs):
    x = np.asarray(x, dtype=np.float32)
    v1 = np.asarray(v1, dtype=np.float32)
    if "nc" not in _CACHE:
        _CACHE["nc"] = build_program()
    nc = _CACHE["nc"]
    in_maps = _host_prep(x, np.asarray(v1), np.asarray(Wq), np.asarray(Wk),
                         np.asarray(Wv), np.asarray(Wproj), np.asarray(Wg),
                         np.float32(lamb))
    res = run_bass_kernel_spmd(nc, in_maps, core_ids=list(range(N_CORES)),
                               **run_kwargs)
    _CACHE["last_results"] = res
    out = np.zeros((B, T, D), dtype=np.float32)
    for c in range(N_CORES):
        b = c // 4
        out[b] += res.results[c]["outT"].T.astype(np.float32)
    return out


# revision 116
# speedup vs baseline: 1.0080x; 1.0026x over previous
"""Causal self-attention (value-residual + QK RMSNorm + RoPE + sigmoid gate)
Trainium2 Bass kernel, sharded over 8 NeuronCores.

Sharding: core c handles batch b = c // 4 and the 4 heads [4*(c%4), 4*(c%4)+4).
Each core computes its heads' QKV, attention and gating, then a partial
c_proj contribution out_partial^T = Wproj[:, islice] @ y_gated^T  [1024, 2048]
in f16; host sums the 4 partials per batch in f32 and transposes back.

Key optimizations over the v1 kernel:
 - QKV projections in fp8 (e4m3) DoubleRow matmuls with 3-term residual
   correction: q = x8@W8 + x8@Wr8 + xr8@W8 (likewise k, v).  Host prepares
   x8/xr8 and W8/Wr8; PE cost drops to 0.75x of the f16 path at full
   precision recovery (error ~ quantization^2).
 - Phase interleaving: attention for q-chunk qc only needs the first
   4*(qc+1) token tiles of phase 1, so phase-1 m-groups are emitted
   between attention chunks and all five engines stay busy.
 - Engine rebalance: PSUM->SBUF copies and v-mix on Pool (gpsimd), RMS
   stats/RoPE/masks/y-scale on DVE, exp-only on ACT, output DMAs issued
   from the ACT queue, input DMAs from SP.
 - y path in bf16 (halves y-transpose cost), c_proj keeps bf16 weights.
 - One fused input DMA per token tile (x8|xr8) plus one for (v1|cos|sin).
 - 4-head y accumulator in one PSUM bank with batched normalization
   (one reciprocal + one gate*rec + one apply per j instead of four).
 - Paired PSUM drains: both transpose halves share one psum tile and are
   drained by a single 256-wide copy; cproj's two half-rounds accumulate
   into one full-bank psum tile with a single 512-wide drain; the four
   gate matmuls share one psum tile (pending-zero writes) with one drain.
 - All cproj/ytp drains on DVE (ACT runs exps only in the steady state);
   early qk-transpose drains (m<12) ride the then-idle ACT.
 - f16 gate path and f16 output staging/DMA (host sums partials in f32).
"""

import sys

sys.path.insert(0, "/opt/trn_rl_repo")

import math

import numpy as np
import ml_dtypes

import concourse.bass as bass
import concourse.mybir as mybir
import concourse.tile as tile
from concourse.bass_utils import run_bass_kernel_spmd
from concourse import bacc


# Force Exp and Ln to resolve to the combined natural_log_exp_and_others set:
# the greedy table-load pass otherwise alternates exp_and_others/natural_log,
# inserting a table load per switch. Entry positions are preserved so
# act_func_set_id indices stay valid.
_orig_gat = bacc.get_activation_tables


def _gat_combined(arch):
    out = {}
    for name, fns in _orig_gat(arch).items():
        if name != "natural_log_exp_and_others":
            fns = {f for f in fns
                   if str(f).split(".")[-1] not in ("Exp", "Ln")}
        out[name] = fns
    return out


bacc.get_activation_tables = _gat_combined

F32 = mybir.dt.float32
F32R = mybir.dt.float32r
F16 = mybir.dt.float16
BF16 = mybir.dt.bfloat16
F8 = mybir.dt.float8e4
AF = mybir.ActivationFunctionType
OP = mybir.AluOpType
DR = mybir.MatmulPerfMode.DoubleRow

B, T, D, H, HD = 2, 2048, 1024, 16, 64
HL = 4            # heads per core
OL = HL * HD      # 256 local qkv width
NT = T // 128     # 16 t-tiles
KD = D // 128     # 8 contraction tiles
NQC = T // 512    # 4 q chunks
EPS = 1.1920929e-7
ATTN_SCALE = 0.1
ROPE_BASE = 10000.0
N_CORES = 8
NP8 = ml_dtypes.float8_e4m3
SX = 32.0         # fp8 scale for x8/xr8 (e4m3 max is 240; |x| < 6)
SW = 64.0         # fp8 scale for W8/Wr8 (|W| <= 1/32)
DEQ = 1.0 / (SX * SW)

_CACHE = {}
PHASE_MARKS = []   # (label, first_instruction_number) in emission order


def _mark(nc, label):
    n = nc._state.get_next_instruction_name()  # consumes one name
    PHASE_MARKS.append((label, int(n.split("-")[1])))


def r32(ap):
    return ap.bitcast(F32R)


def build_program():
    nc = bacc.Bacc("TRN2", target_bir_lowering=False, debug=False, num_devices=1)

    # host-tiled inputs (layouts chosen for single fat DMAs, no elem penalty)
    xx8 = nc.dram_tensor("xx8", [128, NT, 2 * KD * 128], F8,
                         kind="ExternalInput").ap()          # x8 | xr8
    vcs = nc.dram_tensor("vcs", [128, NT, 384], F16,
                         kind="ExternalInput").ap()          # lam*v1 | cos | sin
    WSZ = (KD // 2) * 2 * OL
    wq8 = nc.dram_tensor("wq8", [128, WSZ], F8, kind="ExternalInput").ap()
    wk8 = nc.dram_tensor("wk8", [128, WSZ], F8, kind="ExternalInput").ap()
    wv8 = nc.dram_tensor("wv8", [128, WSZ], F8, kind="ExternalInput").ap()
    wqr8 = nc.dram_tensor("wqr8", [128, WSZ], F8, kind="ExternalInput").ap()
    wkr8 = nc.dram_tensor("wkr8", [128, WSZ], F8, kind="ExternalInput").ap()
    wvr8 = nc.dram_tensor("wvr8", [128, WSZ], F8, kind="ExternalInput").ap()
    wp = nc.dram_tensor("wp", [128, 2, D], BF16, kind="ExternalInput").ap()
    wg = nc.dram_tensor("wg", [12, HL], F16, kind="ExternalInput").ap()
    xg = nc.dram_tensor("xg", [12, T], F16, kind="ExternalInput").ap()
    id16 = nc.dram_tensor("id16", [128, 128], F16, kind="ExternalInput").ap()
    idbf = nc.dram_tensor("idbf", [128, 128], BF16, kind="ExternalInput").ap()
    msk16 = nc.dram_tensor("msk16", [128, 128], F16, kind="ExternalInput").ap()
    outT = nc.dram_tensor("outT", [D, T], F16, kind="ExternalOutput").ap()

    with tile.TileContext(nc) as tc:
        import contextlib
        stack = contextlib.ExitStack()
        pers = stack.enter_context(tc.tile_pool(name="pers", bufs=1))
        xp = stack.enter_context(tc.tile_pool(name="xp", bufs=5))
        sp = stack.enter_context(tc.tile_pool(name="sp", bufs=4))
        pp = stack.enter_context(tc.tile_pool(name="pp", bufs=50))
        op_ = stack.enter_context(tc.tile_pool(name="op", bufs=10))
        yp = stack.enter_context(tc.tile_pool(name="yp", bufs=2))
        # PSUM budget is 8 banks of 2KB; every ring slot is bank-rounded.
        # pss: 2 slots x 2 banks.  psa: tag "ph1" (qkv/transpose/gate psums)
        # and tag "attn" (y/ytranspose/cproj psums), 2 bank-slots each.
        pss = stack.enter_context(tc.tile_pool(name="pss", bufs=2, space="PSUM"))
        psa = stack.enter_context(tc.tile_pool(name="psa", bufs=2, space="PSUM"))

        # ---- persistent tiles / weight DMAs (issue order = need order) ----
        def ptile(shape, dtype, name):
            return pers.tile(shape, dtype, name=name, tag=name, bufs=1)

        def wview(t):
            return t.rearrange("p (c s o) -> p c s o", s=2, o=OL)

        wq8_f = ptile([128, WSZ], F8, "wq8_sb")
        wqr8_f = ptile([128, WSZ], F8, "wqr8_sb")
        wk8_f = ptile([128, WSZ], F8, "wk8_sb")
        wkr8_f = ptile([128, WSZ], F8, "wkr8_sb")
        wv8_f = ptile([128, WSZ], F8, "wv8_sb")
        wvr8_f = ptile([128, WSZ], F8, "wvr8_sb")
        nc.sync.dma_start(out=wq8_f, in_=wq8)
        nc.sync.dma_start(out=wqr8_f, in_=wqr8)

        def kv_dma_hook(nm):
            # called during phase1(0) emission, before the k / v matmuls
            if nm == "k":
                nc.sync.dma_start(out=wk8_f, in_=wk8)
                nc.sync.dma_start(out=wkr8_f, in_=wkr8)
            elif nm == "v":
                nc.sync.dma_start(out=wv8_f, in_=wv8)
                nc.sync.dma_start(out=wvr8_f, in_=wvr8)
        wq8_sb, wqr8_sb = wview(wq8_f), wview(wqr8_f)
        wk8_sb, wkr8_sb = wview(wk8_f), wview(wkr8_f)
        wv8_sb, wvr8_sb = wview(wv8_f), wview(wvr8_f)
        id16_sb = ptile([128, 128], F16, "id16_sb")
        idbf_sb = ptile([128, 128], BF16, "idbf_sb")
        msk_sb = ptile([128, 128], F16, "msk_sb")
        xg_sb = ptile([12, T], F16, "xg_sb")
        wg_sb = ptile([12, HL], F16, "wg_sb")
        nc.sync.dma_start(out=id16_sb, in_=id16)
        wp_sb = ptile([128, 2, D], BF16, "wp_sb")

        def late_dmas():
            nc.sync.dma_start(out=msk_sb, in_=msk16)
            nc.sync.dma_start(out=xg_sb, in_=xg)
            nc.sync.dma_start(out=wg_sb, in_=wg)
            nc.sync.dma_start(out=idbf_sb, in_=idbf)
            nc.sync.dma_start(out=wp_sb, in_=wp)

        eps_sb = ptile([128, 1], F32, "eps_sb")
        nc.vector.memset(eps_sb, EPS)

        v_m = ptile([128, NT, HL, 66], F16, "v_m")
        nc.gpsimd.memset(v_m[:, :, :, 64:66], 1.0)
        gate_all = ptile([128, NT, HL], F32, "gate_all")
        kTt = ptile([128, NT, 2, 128], F16, "kTt")
        qTc = ptile([128, NQC, 2, 512], F16, "qTc")
        ysb = ptile([128, NT, 2, 128], BF16, "ysb")

        rots = [None] * NT

        # ---- phase 1a for one token tile m -------------------------------
        def phase1(m):
            xt = xp.tile([128, 2 * KD * 128], F8, name=f"xx{m}", tag="xx")
            nc.sync.dma_start(out=xt, in_=xx8[:, m])
            vt = xp.tile([128, 384], F16, name=f"vcs{m}", tag="vcs")
            nc.sync.dma_start(out=vt, in_=vcs[:, m])
            x8v = xt[:, 0:1024].rearrange("p (c s t) -> p c s t", s=2, t=128)
            xr8v = xt[:, 1024:2048].rearrange("p (c s t) -> p c s t", s=2, t=128)
            v1t = vt[:, 0:256].rearrange("p (h d) -> p h d", h=HL)
            ctv = (vt[:, 256:320].rearrange("p (a f) -> p a f", a=2)
                   .unsqueeze(2).broadcast_to([128, 2, HL, 32]))
            stv = (vt[:, 320:384].rearrange("p (a f) -> p a f", a=2)
                   .unsqueeze(2).broadcast_to([128, 2, HL, 32]))

            qk_ps = psa.tile([128, 2, OL], F32, name=f"qk_ps{m}", tag="ph1")
            pses = []
            for w8, wr8, nm in ((wq8_sb, wqr8_sb, "q"), (wk8_sb, wkr8_sb, "k"),
                                (wv8_sb, wvr8_sb, "v")):
                if m == 0 and nm != "q":
                    kv_dma_hook(nm)
                if nm == "q":
                    ps = qk_ps[:, 0]
                elif nm == "k":
                    ps = qk_ps[:, 1]
                else:
                    ps = psa.tile([128, OL], F32, name=f"v_ps{m}", tag="ph1")
                nc.tensor.matmul(ps, x8v[:, 0], w8[:, 0], start=True,
                                 stop=False, perf_mode=DR)
                for c in range(1, 4):
                    nc.tensor.matmul(ps, x8v[:, c], w8[:, c], start=False,
                                     stop=False, perf_mode=DR)
                for c in range(4):
                    nc.tensor.matmul(ps, x8v[:, c], wr8[:, c], start=False,
                                     stop=False, perf_mode=DR)
                for c in range(4):
                    nc.tensor.matmul(ps, xr8v[:, c], w8[:, c], start=False,
                                     stop=(c == 3), perf_mode=DR)
                pses.append(ps)
            q_ps, k_ps, v_ps = pses

            # PSUM carries x8@W8-style products scaled by SX*SW = 2048.
            # DVE dequant-copies drain PSUM (Pool cannot touch PSUM, and a
            # DVE op may read at most one PSUM operand).
            qks = sp.tile([128, 2, HL, HD], F16, name=f"qks{m}", tag="qks")
            nc.vector.tensor_scalar_mul(
                qks, qk_ps.rearrange("t a (h d) -> t a h d", h=HL), DEQ)
            nc.vector.scalar_tensor_tensor(
                out=v_m[:, m, :, 0:64],
                in0=v_ps.rearrange("t (h d) -> t h d", h=HL),
                scalar=DEQ, in1=v1t, op0=OP.mult, op1=OP.add)

            qkf = qks.rearrange("t a h d -> t (a h d)")
            scr = sp.tile([128, 2 * OL], F16, name=f"scr{m}", tag="scr")
            nc.vector.tensor_mul(scr, qkf, qkf)
            ss = sp.tile([128, 8], F16, name=f"ss{m}", tag="ss")
            with nc.allow_low_precision(reason="64-term f16 sum feeds rsqrt"):
                nc.vector.tensor_reduce(
                    out=ss, in_=scr.rearrange("t (g d) -> t g d", d=HD),
                    axis=mybir.AxisListType.X, op=OP.add)
            lnv = sp.tile([128, 8], F32, name=f"lnv{m}", tag="lnv")
            nc.scalar.activation(lnv, ss, AF.Ln, scale=1.0 / HD, bias=eps_sb)
            rs = sp.tile([128, 2, HL], F16, name=f"rs{m}", tag="rs")
            # rs = rsqrt(mean q^2); 0.1 for q is folded into host cos/sin.
            nc.scalar.activation(rs.rearrange("t a h -> t (a h)"), lnv,
                                 AF.Exp, scale=-0.5)
            rs2 = sp.tile([128, 2, HL, 2], F16, name=f"rs2{m}", tag="rs2")
            nc.gpsimd.tensor_copy(
                rs2, rs.unsqueeze(3).broadcast_to([128, 2, HL, 2]))
            qk = sp.tile([128, 2, HL, HD], F16, name=f"qk{m}", tag="qk")
            nc.vector.tensor_mul(
                qk.rearrange("t a h (f e) -> t a h f e", e=2), qks.rearrange(
                    "t a h (f e) -> t a h f e", e=2),
                rs2.unsqueeze(3).broadcast_to([128, 2, HL, 32, 2]))

            a_, b_ = qk[:, :, :, 0:32], qk[:, :, :, 32:64]
            rot = sp.tile([128, 2, HL, HD], F16, name=f"rot{m}", tag="rot",
                          bufs=3)
            r1, r2 = rot[:, :, :, 0:32], rot[:, :, :, 32:64]
            tmp = sp.tile([128, 2, HL, 32], F16, name=f"tmp{m}", tag="tmp")
            tmp2 = sp.tile([128, 2, HL, 32], F16, name=f"tmp2{m}", tag="tmp2")
            nc.vector.tensor_mul(tmp, b_, stv)
            nc.vector.tensor_mul(r1, a_, ctv)
            nc.vector.tensor_add(r1, r1, tmp)
            nc.gpsimd.tensor_mul(tmp2, a_, stv)
            nc.vector.tensor_mul(r2, b_, ctv)
            nc.vector.tensor_sub(r2, r2, tmp2)

            rots[m] = rot

        # ---- phase 1b: transposes for tile m (emitted later so they do
        # not block the next tile's QKV matmuls in the in-order PE queue) --
        def phase1b(m):
            # early tiles drain their transposes on the then-idle ACT;
            # both halves share one psum tile -> ONE 256-wide drain copy
            cp = nc.scalar.copy if m < 12 else nc.vector.tensor_copy
            rotf = rots[m].rearrange("t a h d -> t (a h d)")  # q 0-255 k 256-511
            tps = psa.tile([128, 2, 128], F16, name=f"tpq{m}", tag="attn")
            for half in range(2):
                nc.tensor.transpose(tps[:, half],
                                    rotf[:, 128 * half:128 * half + 128],
                                    id16_sb)
            cp(qTc[:, m // 4, :, 128 * (m % 4):128 * (m % 4) + 128], tps)
            tpk = psa.tile([128, 2, 128], F16, name=f"tpk{m}", tag="attn")
            for half in range(2):
                nc.tensor.transpose(tpk[:, half],
                                    rotf[:, 256 + 128 * half:384 + 128 * half],
                                    id16_sb)
            cp(kTt[:, m], tpk)

        # ---- gate for a group of 4 token tiles ---------------------------
        def gate_group(g):
            gg = sp.tile([128, 4, HL], F32, name=f"gg{g}", tag="gg")
            g_ps = psa.tile([128, 4, HL], F32, name=f"g_ps{g}", tag="attn")
            # one zero-region group; starts mark the region pending-zero so
            # each disjoint [*, mi] write replaces rather than accumulates
            for mi in range(4):
                m = 4 * g + mi
                nc.tensor.matmul(g_ps[:, mi], xg_sb[:, 128 * m:128 * m + 128],
                                 wg_sb, start=(mi == 0), stop=(mi == 3))
            nc.vector.tensor_copy(gg, g_ps)
            gge = sp.tile([128, 4, HL], F32, name=f"gge{g}", tag="gge")
            nc.scalar.activation(gge.rearrange("t a h -> t (a h)"),
                                 gg.rearrange("t a h -> t (a h)"),
                                 AF.Exp, scale=-1.0)
            nc.gpsimd.tensor_scalar_add(gge, gge, 1.0)
            nc.vector.reciprocal(
                gate_all[:, 4 * g:4 * g + 4].rearrange("t a h -> t (a h)"),
                gge.rearrange("t a h -> t (a h)"))

        # ---- attention for one q chunk, split into schedulable blocks ----
        pts_all = [[[None] * NT for _ in range(2)] for _ in range(NQC)]

        def att_scores(qc, i0=0, i1=None):
            pts = pts_all[qc]
            for i in range(i0, 4 * qc + 4 if i1 is None else i1):
                ql0 = max(0, 128 * (i - 4 * qc))
                ncols = 512 - ql0
                for p in range(2):
                    s_ps = pss.tile([128, 2, 512], F32, name=f"s{p}_{qc}_{i}",
                                    tag="pss")
                    for r in range(2):
                        prt = slice(64 * r, 64 * r + 64)
                        nc.tensor.matmul(
                            s_ps[:, r, 0:ncols],
                            kTt[prt, i, p],
                            qTc[prt, qc, p, ql0:512],
                            start=True, stop=True,
                            tile_position=(64 * r, 0))
                    pt = pp.tile([128, 2, 512], F16, name=f"pt{p}_{qc}_{i}",
                                 tag="pt")
                    nc.scalar.activation(pt[:, :, ql0:512], s_ps[:, :, 0:ncols],
                                         AF.Exp)
                    if i >= 4 * qc:
                        mb = msk_sb.unsqueeze(1).broadcast_to([128, 2, 128])
                        mul = nc.vector.tensor_mul if qc >= 2 \
                            else nc.gpsimd.tensor_mul
                        mul(pt[:, :, ql0:ql0 + 128],
                            pt[:, :, ql0:ql0 + 128], mb)
                    pts[p][i] = pt

        def att_pv(qc, js0=0, js1=4):
            # 4-head y accumulator in ONE psum bank; batched normalize:
            # rec4 = 1/rowsum (DVE), rg = rec*gate (Pool), one apply (DVE).
            pts = pts_all[qc]
            for js in range(js0, js1):
                j = 4 * qc + js
                y_ps = psa.tile([128, HL, 66], F32, name=f"y_{j}", tag="attn")
                for p in range(2):
                    for r in range(2):
                        h = 2 * p + r
                        for i in range(j + 1):
                            nc.tensor.matmul(
                                y_ps[:, h],
                                pts[p][i][:, r, 128 * js:128 * js + 128],
                                v_m[:, i, h, 0:66],
                                start=(i == 0), stop=(i == j))
                rec4 = sp.tile([128, HL], F32, name=f"rec{j}", tag="rec")
                nc.vector.reciprocal(
                    rec4, y_ps[:, :, 64:65].rearrange("t h o -> t (h o)"))
                rg = sp.tile([128, HL], F32, name=f"rg{j}", tag="rg")
                nc.vector.tensor_mul(rg, rec4, gate_all[:, j])
                ysb_j = (ysb[:, j].rearrange("t p d -> t (p d)")
                         .rearrange("t (h e) -> t h e", h=HL))
                nc.vector.tensor_mul(
                    ysb_j, y_ps[:, :, 0:64],
                    rg.unsqueeze(2).broadcast_to([128, HL, 64]))

        yTq_c = [None] * NQC
        ots = [None] * 8

        def ensure_yT(qc):
            if yTq_c[qc] is None:
                yTq_c[qc] = yp.tile([128, 2, 512], BF16, name=f"yT{qc}",
                                    tag="yT")
            return yTq_c[qc]

        def att_ytp(qc, js):
            # PE-transpose ysb[j] into yTq columns (+DVE drains)
            yTq = ensure_yT(qc)
            j = 4 * qc + js
            for half in range(2):
                ytp = psa.tile([128, 128], BF16, name=f"ty{j}{half}",
                               tag="attn")
                nc.tensor.transpose(ytp, ysb[:, j, half], idbf_sb)
                nc.vector.tensor_copy(yTq[:, half, 128 * js:128 * js + 128],
                                      ytp)

        def att_pv_j(qc, js, i0, i1, stop):
            # partial PV chain for tier js: i in [i0, i1); psum group left
            # open unless stop. Caller finishes with att_pv_fin.
            pts = pts_all[qc]
            j = 4 * qc + js
            if i0 == 0:
                pv_ps[(qc, js)] = psa.tile([128, HL, 66], F32,
                                           name=f"y_{j}", tag="ph1")
            y_ps = pv_ps[(qc, js)]
            for p in range(2):
                for r in range(2):
                    h = 2 * p + r
                    for i in range(i0, i1):
                        nc.tensor.matmul(
                            y_ps[:, h],
                            pts[p][i][:, r, 128 * js:128 * js + 128],
                            v_m[:, i, h, 0:66],
                            start=(i == 0), stop=(stop and i == i1 - 1))

        def att_pv_fin(qc, js):
            j = 4 * qc + js
            y_ps = pv_ps[(qc, js)]
            rec4 = sp.tile([128, HL], F32, name=f"rec{j}", tag="rec")
            nc.vector.reciprocal(
                rec4, y_ps[:, :, 64:65].rearrange("t h o -> t (h o)"))
            rg = sp.tile([128, HL], F32, name=f"rg{j}", tag="rg")
            nc.gpsimd.tensor_mul(rg, rec4, gate_all[:, j])
            ysb_j = (ysb[:, j].rearrange("t p d -> t (p d)")
                     .rearrange("t (h e) -> t h e", h=HL))
            nc.vector.tensor_mul(
                ysb_j, y_ps[:, :, 0:64],
                rg.unsqueeze(2).broadcast_to([128, HL, 64]))

        pv_ps = {}

        def att_cproj(qc, tc2, dma):
            yTq = ensure_yT(qc)
            lsl = slice(256 * tc2, 256 * tc2 + 256)
            tsl = slice(512 * qc, 512 * qc + 512)
            for oc in range(8):
                osl = slice(128 * oc, 128 * oc + 128)
                if tc2 == 0:
                    ots[oc] = op_.tile([128, 512], F16,
                                       name=f"ot{oc}_{qc}", tag="ot")
                pr = psa.tile([128, 256], F32, name=f"pr{oc}_{qc}_{tc2}",
                              tag="attn")
                for a in range(2):
                    nc.tensor.matmul(pr, wp_sb[:, a, osl],
                                     yTq[:, a, lsl],
                                     start=(a == 0), stop=(a == 1))
                nc.vector.tensor_copy(ots[oc][:, lsl], pr)
                if dma:
                    nc.sync.dma_start(out=outT[osl, tsl], in_=ots[oc])

        def att_tail(qc, part=None):
            # tail drains ride ACT in round 0/1 (ACT slack); DVE otherwise.
            # qc3 (post-last-exp, ACT idle): alternate copies ACT/DVE so the
            # final drain runs on both engines in parallel.
            on_act = False
            ot_copy = nc.scalar.copy if on_act else nc.vector.tensor_copy
            ytp_copy = nc.vector.tensor_copy
            if yTq_c[qc] is None:
                yTq_c[qc] = yp.tile([128, 2, 512], BF16, name=f"yT{qc}",
                                    tag="yT")
            yTq = yTq_c[qc]
            parts = (0, 1) if part is None else (part,)
            for tc2 in parts:
                for js in (2 * tc2, 2 * tc2 + 1):
                    j = 4 * qc + js
                    ytp = psa.tile([128, 2, 128], BF16, name=f"ty{j}",
                                   tag="attn")
                    for half in range(2):
                        nc.tensor.transpose(ytp[:, half], ysb[:, j, half],
                                            idbf_sb)
                    ytp_copy(yTq[:, :, 128 * js:128 * js + 128], ytp)
            if part is None:
                tsl = slice(512 * qc, 512 * qc + 512)
                for oc in range(8):
                    osl = slice(128 * oc, 128 * oc + 128)
                    ot = op_.tile([128, 512], F16, name=f"ot{oc}_{qc}",
                                  tag="ot")
                    # qc3: odd oc borrow the idle ph1 psum ring -> 4
                    # cproj rounds in flight instead of 2 at the very end
                    ptag = "ph1" if (qc == 3 and oc % 2 == 1) else "attn"
                    pr = psa.tile([128, 512], F32, name=f"pr{oc}_{qc}",
                                  tag=ptag)
                    for tc2 in range(2):
                        lsl = slice(256 * tc2, 256 * tc2 + 256)
                        for a in range(2):
                            nc.tensor.matmul(pr[:, lsl], wp_sb[:, a, osl],
                                             yTq[:, a, lsl],
                                             start=(a == 0), stop=(a == 1))
                    cp = (nc.scalar.copy if (qc == 3 and oc % 2 == 1)
                          else ot_copy)
                    cp(ot, pr)
                    nc.sync.dma_start(out=outT[osl, tsl], in_=ot)
            else:
                # half-tail: fill ot[:, half]; the tc2=1 half emits the DMA
                tc2 = part
                lsl = slice(256 * tc2, 256 * tc2 + 256)
                tsl = slice(512 * qc, 512 * qc + 512)
                for oc in range(8):
                    osl = slice(128 * oc, 128 * oc + 128)
                    if tc2 == 0:
                        ots[oc] = op_.tile([128, 512], F16,
                                           name=f"ot{oc}_{qc}", tag="ot")
                    pr = psa.tile([128, 256], F32, name=f"pr{oc}_{qc}_{tc2}",
                                  tag="attn")
                    for a in range(2):
                        nc.tensor.matmul(pr, wp_sb[:, a, osl],
                                         yTq[:, a, lsl],
                                         start=(a == 0), stop=(a == 1))
                    ot_copy(ots[oc][:, lsl], pr)
                    if tc2 == 1:
                        nc.sync.dma_start(out=outT[osl, tsl], in_=ots[oc])

        # ---- schedule (software-pipelined; per-engine queues are in-order
        # so later-dependent PE work is always emitted after independent
        # PE work that can run now) ----------------------------------------
        # ---- schedule: phase-1 tiles staggered so two QKV tile-loads run
        # ahead of the chain, and the next group's first tiles are issued
        # before each score round to keep PE fed during chain tails. -------
        sched = """
        phase1(0)
        phase1(1)
        late_dmas()
        phase1b(0); phase1(2)
        phase1b(1); phase1(3)
        phase1b(2); gate_group(0)
        phase1(4); phase1b(3)
        att_scores(0, 0, 1); phase1(5)
        att_scores(0, 1, 2); phase1b(4)
        att_scores(0, 2, 3); phase1(6)
        att_scores(0, 3, 4); phase1b(5); phase1(7)
        phase1b(6); gate_group(1)
        phase1(8); phase1b(7); phase1(9)
        att_scores(1, 0, 1); phase1b(8)
        att_scores(1, 1, 2); phase1(10)
        att_scores(1, 2, 3); phase1b(9)
        att_scores(1, 3, 4); phase1(11)
        att_pv(0, 0, 2)
        att_scores(1, 4, 5); phase1b(10)
        att_pv(0, 2, 4)
        att_scores(1, 5, 6); gate_group(2)
        att_scores(1, 6, 7); phase1(12)
        att_scores(1, 7, 8); phase1b(11); phase1(13)
        att_tail(0)
        att_scores(2, 0, 1); phase1b(12)
        att_scores(2, 1, 2); phase1(14)
        att_scores(2, 2, 3); phase1b(13)
        att_scores(2, 3, 4); phase1(15)
        att_scores(2, 4, 6); phase1b(14)
        att_pv(1, 0, 2)
        att_scores(2, 6, 8); gate_group(3)
        att_pv(1, 2, 4)
        att_scores(2, 8, 10); phase1b(15)
        att_scores(2, 10, 12)
        att_tail(1)
        att_scores(3, 0, 4)
        att_pv(2, 0, 2)
        att_scores(3, 4, 8)
        att_pv(2, 2, 4)
        att_scores(3, 8, 13)
        att_scores(3, 13, 14)
        att_tail(2)
        att_pv(3, 0, 1)
        att_scores(3, 14, 15)
        att_scores(3, 15, 16)
        att_pv(3, 1, 2)
        att_pv(3, 2, 3)
        att_pv(3, 3, 4)
        att_tail(3)
        """
        env = locals()
        for line in sched.strip().split("\n"):
            for call in line.strip().split("; "):
                call = call.strip()
                if not call:
                    continue
                _mark(nc, call.replace(" ", ""))
                eval(call, {}, env)

        stack.close()

    nc.compile()
    return nc


def _host_prep(x, v1, Wq, Wk, Wv, Wproj, Wg, lamb):
    lamb = np.float32(lamb)
    half = HD // 2
    inv_freq = 1.0 / (ROPE_BASE ** (np.arange(0, HD, 2, dtype=np.float32) / HD))
    freqs = np.arange(T, dtype=np.float32)[:, None] * inv_freq[None, :]  # [T, 32]
    cosf = np.cos(freqs).astype(np.float32)
    sinf = np.sin(freqs).astype(np.float32)

    id16 = np.eye(128, dtype=np.float16)
    idbf = np.eye(128, dtype=ml_dtypes.bfloat16)
    # causal keep-mask in S^T layout: rows=k_local, cols=q_local; keep q >= k
    kk, qq = np.meshgrid(np.arange(128), np.arange(128), indexing="ij")
    msk16 = (qq >= kk).astype(np.float16)

    def quant_pair(w):
        w8 = (w * SW).astype(NP8)
        wr8 = (w * SW - w8.astype(np.float32)).astype(NP8)
        return w8, wr8

    def wtile(w8):
        # [D, OL] -> [128, (KD//2)*2*OL] with [p, (c,s,o)] = w8[(2c+s)*128+p, o]
        return np.ascontiguousarray(
            w8.reshape(KD // 2, 2, 128, OL).transpose(2, 0, 1, 3)
            .reshape(128, -1))

    WqT = np.ascontiguousarray(Wq.T).astype(np.float32)
    WkT = np.ascontiguousarray(Wk.T).astype(np.float32)
    WvT = (np.ascontiguousarray(Wv.T) * (1.0 - lamb)).astype(np.float32)

    in_maps = []
    for c in range(N_CORES):
        b, g = c // 4, c % 4
        hsl = slice(g * OL, (g + 1) * OL)

        # x8 / xr8 tiled [128, NT, 2*1024]
        xb = x[b].astype(np.float32) * SX     # [T, D]
        x8 = xb.astype(NP8)
        xr8 = (xb - x8.astype(np.float32)).astype(NP8)

        def xtile(x8_):
            # [T, D] -> [128p(feat), NT, KD*128] : [p, m, kd*128+t']
            # = x8_[128m+t', 128kd+p]
            a = x8_.reshape(NT, 128, KD, 128)        # [m, t', kd, p]
            return a.transpose(3, 0, 2, 1).reshape(128, NT, KD * 128)

        xx8 = np.concatenate([xtile(x8), xtile(xr8)], axis=2)  # [128, NT, 2048]

        # vcs: [128, NT, 384] f16 : lam*v1 (256) | cos pair (64) | sin pair (64)
        v1c = (v1[b, :, g * HL:(g + 1) * HL, :].astype(np.float32) * lamb)
        v1t = v1c.reshape(NT, 128, OL).transpose(1, 0, 2)      # [t', m, 256]
        cs = np.empty((128, NT, 2, 2, 32), dtype=np.float32)
        cosr = cosf.reshape(NT, 128, 32).transpose(1, 0, 2)    # [t', m, 32]
        sinr = sinf.reshape(NT, 128, 32).transpose(1, 0, 2)
        cs[:, :, 0, 0] = 0.1 * cosr
        cs[:, :, 0, 1] = 0.1 * sinr
        cs[:, :, 1, 0] = cosr
        cs[:, :, 1, 1] = sinr
        # kernel reads ctv = [256:320] = [a, 32] cos ; stv = [320:384] sin
        vcs = np.empty((128, NT, 384), dtype=np.float16)
        vcs[:, :, 0:256] = v1t
        vcs[:, :, 256:288] = cs[:, :, 0, 0]
        vcs[:, :, 288:320] = cs[:, :, 1, 0]
        vcs[:, :, 320:352] = cs[:, :, 0, 1]
        vcs[:, :, 352:384] = cs[:, :, 1, 1]

        q8, qr8 = quant_pair(WqT[:, hsl])
        k8, kr8 = quant_pair(WkT[:, hsl])
        v8, vr8 = quant_pair(WvT[:, hsl])

        in_maps.append({
            "xx8": np.ascontiguousarray(xx8),
            "vcs": np.ascontiguousarray(vcs),
            "wq8": wtile(q8), "wqr8": wtile(qr8),
            "wk8": wtile(k8), "wkr8": wtile(kr8),
            "wv8": wtile(v8), "wvr8": wtile(vr8),
            "wp": np.ascontiguousarray(
                Wproj[:, hsl].T.reshape(2, 128, D).transpose(1, 0, 2)
            ).astype(ml_dtypes.bfloat16),
            "wg": np.ascontiguousarray(Wg[g * HL:(g + 1) * HL, :].T)
                 .astype(np.float16),
            "xg": np.ascontiguousarray(x[b, :, :12].T).astype(np.float16),
            "id16": id16, "idbf": idbf, "msk16": msk16,
        })
    return in_maps


def kernel(x, v1, Wq, Wk, Wv, Wproj, Wg, lamb, **run_kwargs):
    x = np.asarray(x, dtype=np.float32)
    v1 = np.asarray(v1, dtype=np.float32)
    if "nc" not in _CACHE:
        _CACHE["nc"] = build_program()
    nc = _CACHE["nc"]
    in_maps = _host_prep(x, np.asarray(v1), np.asarray(Wq), np.asarray(Wk),
                         np.asarray(Wv), np.asarray(Wproj), np.asarray(Wg),
                         np.float32(lamb))
    res = run_bass_kernel_spmd(nc, in_maps, core_ids=list(range(N_CORES)),
                               **run_kwargs)
    _CACHE["last_results"] = res
    out = np.zeros((B, T, D), dtype=np.float32)
    for c in range(N_CORES):
        b = c // 4
        out[b] += res.results[c]["outT"].T.astype(np.float32)
    return out
